# revision 5
# baseline (speedup 1.0000x reference)
"""ViT-style dense transformer (12 blocks, dim 768, 199 tokens, B=32) on 8
Trainium2 NeuronCores.

Sharding: data-parallel over batch — 4 images per core, no collectives.

Device layout: activations are kept channels-major (x.T, shape [768, tokens])
so every GEMM uses the weight as the stationary operand directly and no
activation transposes are needed (v is PE-transposed to tokens-major for the
attention-value matmul). LayerNorm statistics (partition-dim sums) are
computed with ones-vector matmuls on the PE; softmax runs over the partition
dim (keys) so the sigmoid loc-mask is a per-partition broadcast. Matmuls run
in float32r (full-rate fp32 path for free-dim >= 256) with fp32 PSUM
accumulation. LayerNorm affine params are folded into the following weight
matrices host-side; biases enter PSUM as K=1 bias-row matmuls.
"""
import contextlib
import os
import sys

sys.path.insert(0, "/opt/trn_rl_repo")

import numpy as np

import concourse.bass as bass
import concourse.tile as tile
from concourse import bacc, mybir
from concourse.bass_utils import run_bass_kernel_spmd
from concourse.masks import make_identity

F32 = mybir.dt.float32
F32R = mybir.dt.float32r
AF = mybir.ActivationFunctionType
OP = mybir.AluOpType

N_CORES = 8
BL = 4            # samples per core
DEPTH = int(os.environ.get("KDEPTH", "12"))
MASK_START = int(os.environ.get("KMASKSTART", "9"))
HEADS, DIM, HD = 12, 768, 64
SCALE = HD ** -0.5
NTOK = 199        # tokens per sample
T = BL * NTOK     # 796 tokens per core
QPAD = 256        # padded query width for fp32r-fast score matmuls
W = T + (QPAD - NTOK)  # 853: q reads for the last sample spill into pad cols
CT = DIM // 128   # 6 channel tiles
CH = (398, 398)   # token chunks for N<=512 matmuls
FCH = (512, 284)  # token chunks for the fused MLP (fc2 psum = 6 banks + fc1 2)
EPS = 1e-5


def chunk_off(c):
    return sum(CH[:c])


def _ln_stats(nc, ps, smpool, stpool, ones, invc, eps_t, x, xsq, c):
    """Per-token mean and rstd (both PSUM [128, cw] broadcasts) over the
    channel (partition x ct) axis of channels-major x, for token chunk c."""
    mm = nc.tensor.matmul
    act = nc.scalar.activation
    tt = nc.vector.tensor_tensor
    co, cw = chunk_off(c), CH[c]
    sraw = ps.tile([1, 398], F32, tag="ps")
    ssraw = ps.tile([1, 398], F32, tag="ps")
    for ct in range(CT):
        mm(sraw[:, :cw], ones[:, 0:1], x[:, ct, co:co + cw],
           start=(ct == 0), stop=(ct == CT - 1))
        mm(ssraw[:, :cw], ones[:, 0:1], xsq[:, ct, co:co + cw],
           start=(ct == 0), stop=(ct == CT - 1))
    srow = smpool.tile([1, 2, 398], F32R, tag="srow")
    act(srow[:, 0, :cw], sraw[:, :cw], AF.Copy)
    act(srow[:, 1, :cw], ssraw[:, :cw], AF.Copy)
    mu = ps.tile([128, 398], F32, tag="ps")
    msq = ps.tile([128, 398], F32, tag="ps")
    mm(mu[:, :cw], invc[:], srow[:, 0, :cw], start=True, stop=True)
    mm(msq[:, :cw], invc[:], srow[:, 1, :cw], start=True, stop=True)
    musq = stpool.tile([128, 398], F32, tag="lnsc")
    act(musq[:, :cw], mu[:, :cw], AF.Square)
    var = stpool.tile([128, 398], F32, tag="lnsc")
    tt(var[:, :cw], msq[:, :cw], musq[:, :cw], op=OP.subtract)
    sd = stpool.tile([128, 398], F32, tag="lnsc")
    act(sd[:, :cw], var[:, :cw], AF.Sqrt, bias=eps_t[:, 0:1])
    rstd = ps.tile([128, 398], F32, tag="ps")
    nc.vector.reciprocal(rstd[:, :cw], sd[:, :cw])
    return mu, rstd


def _ln_apply(nc, ps, smpool, stpool, ones, invc, eps_t, x, xsq, h, out_dt_hint=None):
    """h = (x - mu) * rstd, channels-major, chunk at a time."""
    tt = nc.vector.tensor_tensor
    for c in range(2):
        co, cw = chunk_off(c), CH[c]
        mu, rstd = _ln_stats(nc, ps, smpool, stpool, ones, invc, eps_t, x, xsq, c)
        for ct in range(CT):
            tt(h[:, ct, co:co + cw], x[:, ct, co:co + cw], mu[:, :cw],
               op=OP.subtract)
            tt(h[:, ct, co:co + cw], h[:, ct, co:co + cw], rstd[:, :cw],
               op=OP.mult)


def build_program():
    nc = bacc.Bacc("TRN2", target_bir_lowering=False, debug=False,
                   num_devices=N_CORES)

    # ---- DRAM parameters (per-core views, host-prepped) ----
    d_xT = nc.dram_tensor("xT", [DIM, BL * 196], F32R, kind="ExternalInput")
    d_pw = nc.dram_tensor("patch_wT", [CT, DIM, 128], F32R, kind="ExternalInput")
    d_qkvw = nc.dram_tensor("qkv_wp", [DEPTH, 18, DIM, 128], F32R,
                            kind="ExternalInput")
    d_bias = nc.dram_tensor("biasT", [DEPTH, 128, 54], F32, kind="ExternalInput")
    d_prw = nc.dram_tensor("proj_wp", [DEPTH, CT, DIM, 128], F32R,
                           kind="ExternalInput")
    d_f1w = nc.dram_tensor("fc1_wp", [DEPTH, 24, DIM, 128], F32R,
                           kind="ExternalInput")
    d_f2w = nc.dram_tensor("fc2_w", [DEPTH, 4 * DIM, DIM], F32R,
                           kind="ExternalInput")
    d_ones = nc.dram_tensor("ones_c", [128, 512], F32R, kind="ExternalInput")
    d_invc = nc.dram_tensor("invc_c", [1, 128], F32R, kind="ExternalInput")
    d_zpad = nc.dram_tensor("zpad", [128, 12 * (W - T)], F32R, kind="ExternalInput")
    d_posc = nc.dram_tensor("posc", [DIM, 196], F32, kind="ExternalInput")
    d_extra = nc.dram_tensor("extra_cols", [DIM, 3], F32, kind="ExternalInput")
    d_fing = nc.dram_tensor("final_g", [128, CT], F32, kind="ExternalInput")
    d_finb = nc.dram_tensor("final_b", [128, CT], F32, kind="ExternalInput")
    d_out = nc.dram_tensor("out", [T, DIM], F32, kind="ExternalOutput")

    mm = nc.tensor.matmul
    act = nc.scalar.activation
    tt = nc.vector.tensor_tensor
    ts = nc.vector.tensor_scalar

    KREPEAT = int(os.environ.get("KREPEAT", "1"))
    with tile.TileContext(nc) as tc:
        rep = contextlib.ExitStack()
        if KREPEAT > 1:
            rep.enter_context(tc.For_i(0, KREPEAT, 1))
        with (
            rep,
            tc.tile_pool(name="const", bufs=1) as cpool,
            tc.tile_pool(name="x", bufs=1) as xpool,
            tc.tile_pool(name="big", bufs=2) as bigpool,    # xsq/h/vT/oT/h2 rotate
            tc.tile_pool(name="qk", bufs=1) as qkpool,
            tc.tile_pool(name="v", bufs=1) as vpool,
            tc.tile_pool(name="aa", bufs=1) as aapool,
            tc.tile_pool(name="rr", bufs=1) as rrpool,
            tc.tile_pool(name="wmt", bufs=4) as wmtpool,    # [128,6,128] mt-slices
            tc.tile_pool(name="wkt", bufs=3) as wktpool,    # [128,768] k-slices
            tc.tile_pool(name="bias", bufs=1) as biaspool,
            tc.tile_pool(name="gelu", bufs=3) as gelupool,
            tc.tile_pool(name="stats", bufs=2) as stpool,
            tc.tile_pool(name="small", bufs=1) as smpool,
            tc.tile_pool(name="obuf", bufs=2) as obpool,
            tc.tile_pool(name="ps", bufs=8, space="PSUM") as ps,
        ):
            # ---------------- constants ----------------
            ones = cpool.tile([128, 512], F32R, tag="ones")
            nc.sync.dma_start(ones[:], d_ones[:])
            invc = cpool.tile([1, 128], F32R, tag="invc")
            nc.sync.dma_start(invc[:], d_invc[:])
            eps_t = cpool.tile([128, 1], F32, tag="eps")
            nc.vector.memset(eps_t[:], EPS)
            ident = cpool.tile([128, 128], F32, tag="ident")
            make_identity(nc, ident[:])
            fing = cpool.tile([128, CT], F32, tag="fing")
            nc.sync.dma_start(fing[:], d_fing[:])
            finb = cpool.tile([128, CT], F32, tag="finb")
            nc.sync.dma_start(finb[:], d_finb[:])

            # residual stream, channels-major: x[p, ct, tok]
            x = xpool.tile([128, CT, T], F32R, tag="x")

            # ---------------- patch embed ----------------
            with tc.tile_pool(name="patch", bufs=1) as ppool:
                posc = ppool.tile([128, CT, 196], F32, tag="posc")
                nc.sync.dma_start(posc[:],
                                  d_posc.rearrange("(ct p) t -> p ct t", p=128))
                extra = ppool.tile([128, CT, 3], F32, tag="extra")
                nc.sync.dma_start(extra[:],
                                  d_extra.rearrange("(ct p) t -> p ct t", p=128))
                xt = bigpool.tile([128, CT, BL * 196], F32R, tag="big")
                nc.sync.dma_start(xt[:], d_xT.rearrange("(kt p) t -> p kt t", p=128))
                for mt in range(CT):
                    pw = wmtpool.tile([128, CT, 128], F32R, tag="wmt")
                    nc.sync.dma_start(
                        pw[:], d_pw[mt].rearrange("(kt p) m -> p kt m", p=128))
                    for c in range(2):  # 392-token halves: samples (2c, 2c+1)
                        acc = ps.tile([128, 392], F32, tag="ps")
                        for kt in range(CT):
                            mm(acc[:], pw[:, kt, :],
                               xt[:, kt, c * 392:(c + 1) * 392],
                               start=(kt == 0), stop=(kt == CT - 1))
                        for si in range(2):
                            s = 2 * c + si
                            tt(x[:, mt, s * NTOK + 1: s * NTOK + 197],
                               acc[:, si * 196:(si + 1) * 196], posc[:, mt, :],
                               op=OP.add)
                for s in range(BL):
                    o = s * NTOK
                    nc.vector.tensor_copy(x[:, :, o:o + 1], extra[:, :, 0:1])
                    nc.vector.tensor_copy(x[:, :, o + 197:o + 199],
                                          extra[:, :, 1:3])

            # ---------------- transformer blocks ----------------
            for i in range(DEPTH):
                # ===== LN1 -> h (affine folded into qkv weights) =====
                xsq = bigpool.tile([128, CT, T], F32R, tag="big")
                act(xsq[:], x[:], AF.Square)
                h = bigpool.tile([128, CT, T], F32R, tag="big")
                _ln_apply(nc, ps, smpool, stpool, ones, invc, eps_t, x, xsq, h)

                # ===== qkv = h @ Wqkv (channels-major out) =====
                qk = qkpool.tile([128, 12, W], F32R, tag="qk")
                nc.sync.dma_start(qk[:, :, T:W],
                                  d_zpad.rearrange("p (j t) -> p j t", j=12))
                vT = bigpool.tile([128, CT, T], F32, tag="big")
                bt = biaspool.tile([128, 54], F32, tag="bt")
                nc.sync.dma_start(bt[:], d_bias[i])
                for mt in range(18):
                    wq = wmtpool.tile([128, CT, 128], F32R, tag="wmt")
                    nc.sync.dma_start(
                        wq[:], d_qkvw[i, mt].rearrange("(kt p) m -> p kt m", p=128))
                    for c in range(2):
                        co, cw = chunk_off(c), CH[c]
                        acc = ps.tile([128, 398], F32, tag="ps")
                        for kt in range(CT):
                            mm(acc[:, :cw], wq[:, kt, :], h[:, kt, co:co + cw],
                               start=(kt == 0), stop=(kt == CT - 1))
                        dst = (qk[:, mt, co:co + cw] if mt < 12
                               else vT[:, mt - 12, co:co + cw])
                        act(dst, acc[:, :cw], AF.Identity, bias=bt[:, mt:mt + 1])

                # ===== transpose v to tokens-major v0/v1 =====
                v0 = vpool.tile([128, BL, DIM], F32R, tag="v0")
                v1 = vpool.tile([128, BL, DIM], F32R, tag="v1")
                for s in range(BL):
                    o = s * NTOK
                    for cc, (vt, mw) in enumerate(((v0, 128), (v1, 71))):
                        for ct in range(CT):
                            ptr = ps.tile([128, 128], F32, tag="ps")
                            nc.tensor.transpose(
                                ptr[:mw, :],
                                vT[:, ct, o + cc * 128: o + cc * 128 + mw],
                                ident[:])
                            act(vt[:mw, s, ct * 128:(ct + 1) * 128],
                                ptr[:mw, :], AF.Copy)

                # ===== attention =====
                oT = bigpool.tile([128, CT, T], F32R, tag="big")
                use_mask = i >= MASK_START
                for s in range(BL):
                    o = s * NTOK
                    aa = aapool.tile([128, 12, QPAD], F32R, tag="aa")
                    ab2 = aapool.tile([128, 12, QPAD], F32R, tag="ab2")
                    if use_mask:
                        macc0 = smpool.tile([128, 2], F32, tag="macc0")
                        macc1 = smpool.tile([128, 2], F32, tag="macc1")
                    for h_ in range(12):
                        ro = (h_ % 2) * 64
                        jq, jk = h_ // 2, 6 + h_ // 2
                        ssum = ps.tile([1, QPAD], F32, tag="ps")
                        for cc, mw in ((0, 128), (1, 71)):
                            S = ps.tile([128, QPAD], F32, tag="ps")
                            mm(S[:mw, :],
                               qk[ro:ro + 64, jk, o + cc * 128: o + cc * 128 + mw],
                               qk[ro:ro + 64, jq, o:o + QPAD],
                               start=True, stop=True)
                            at = (aa, ab2)[cc]
                            act(at[:mw, h_, :], S[:mw, :], AF.Exp, scale=SCALE)
                            if use_mask:
                                macc = (macc0, macc1)[cc]
                                if h_ == 0:
                                    nc.vector.tensor_copy(macc[:mw, :],
                                                          S[:mw, 197:199])
                                else:
                                    tt(macc[:mw, :], macc[:mw, :],
                                       S[:mw, 197:199], op=OP.add)
                            mm(ssum[:, :], ones[:mw, 0:1], at[:mw, h_, :],
                               start=(cc == 0), stop=(cc == 1))
                        rr = rrpool.tile([1, QPAD], F32R, tag="rr")
                        with nc.allow_low_precision(reason="f32r is 4-byte"):
                            nc.vector.reciprocal(rr[:], ssum[:])
                        rb = ps.tile([128, QPAD], F32, tag="ps")
                        mm(rb[:], ones[0:1, 0:128], rr[:], start=True, stop=True)
                        tt(aa[:, h_, :], aa[:, h_, :], rb[:128, :], op=OP.mult)
                        tt(ab2[:71, h_, :], ab2[:71, h_, :], rb[:71, :],
                           op=OP.mult)
                    if use_mask:
                        for cc, mw in ((0, 128), (1, 71)):
                            macc = (macc0, macc1)[cc]
                            sg = smpool.tile([128, 2], F32, tag="sg")
                            act(sg[:mw, :], macc[:mw, :], AF.Sigmoid,
                                scale=SCALE / 12)
                            mk = smpool.tile([128, 1], F32, tag="mk")
                            tt(mk[:mw, :], sg[:mw, 0:1], sg[:mw, 1:2], op=OP.max)
                            at = (aa, ab2)[cc]
                            for h_ in range(12):
                                ts(at[:mw, h_, :], at[:mw, h_, :], mk[:mw, 0:1],
                                   None, op0=OP.mult)
                    for h_ in range(12):
                        ro, j = (h_ % 2) * 64, h_ // 2
                        Oh = ps.tile([64, QPAD], F32, tag="ps")
                        mm(Oh[:, :], v0[:, s, h_ * 64:(h_ + 1) * 64],
                           aa[:, h_, :], start=True, stop=False)
                        mm(Oh[:, :], v1[:71, s, h_ * 64:(h_ + 1) * 64],
                           ab2[:71, h_, :], start=False, stop=True)
                        if ro == 0:
                            act(oT[0:64, j, o:o + NTOK], Oh[:, :NTOK], AF.Copy)
                        else:
                            stg = smpool.tile([64, NTOK], F32R, tag="stg")
                            act(stg[:, :], Oh[:, :NTOK], AF.Copy)
                            nc.sync.dma_start(oT[64:128, j, o:o + NTOK], stg[:, :])

                # ===== proj + residual =====
                for mt in range(CT):
                    wp = wmtpool.tile([128, CT, 128], F32R, tag="wmt")
                    nc.sync.dma_start(
                        wp[:], d_prw[i, mt].rearrange("(kt p) m -> p kt m", p=128))
                    for c in range(2):
                        co, cw = chunk_off(c), CH[c]
                        acc = ps.tile([128, 398], F32, tag="ps")
                        for kt in range(CT):
                            mm(acc[:, :cw], wp[:, kt, :], oT[:, kt, co:co + cw],
                               start=(kt == 0), stop=(kt == CT - 1))
                        nc.vector.scalar_tensor_tensor(
                            x[:, mt, co:co + cw], acc[:, :cw],
                            bt[:, 18 + mt:19 + mt], x[:, mt, co:co + cw],
                            op0=OP.add, op1=OP.add)

                # ===== LN2 -> h2 =====
                xsq = bigpool.tile([128, CT, T], F32R, tag="big")
                act(xsq[:], x[:], AF.Square)
                h2 = bigpool.tile([128, CT, T], F32R, tag="big")
                _ln_apply(nc, ps, smpool, stpool, ones, invc, eps_t, x, xsq, h2)

                # ===== MLP: fc1 -> gelu -> fc2 (streamed, fc2 psum resident) ====
                for c in range(2):
                    co = sum(FCH[:c])
                    cw = FCH[c]
                    fc2acc = []
                    for mt in range(CT):
                        a2 = ps.tile([128, 512], F32, tag="ps")
                        fc2acc.append(a2)
                    for gmt in range(24):
                        wf = wmtpool.tile([128, CT, 128], F32R, tag="wmt")
                        nc.sync.dma_start(
                            wf[:], d_f1w[i, gmt].rearrange("(kt p) m -> p kt m",
                                                           p=128))
                        a1 = ps.tile([128, 512], F32, tag="ps")
                        for kt in range(CT):
                            mm(a1[:, :cw], wf[:, kt, :], h2[:, kt, co:co + cw],
                               start=(kt == 0), stop=(kt == CT - 1))
                        g = gelupool.tile([128, 512], F32R, tag="gelu")
                        act(g[:, :cw], a1[:, :cw], AF.Gelu,
                            bias=bt[:, 30 + gmt:31 + gmt])
                        wf2 = wktpool.tile([128, DIM], F32R, tag="wkt")
                        nc.sync.dma_start(
                            wf2[:], d_f2w[i, gmt * 128:(gmt + 1) * 128, :])
                        for mt in range(CT):
                            mm(fc2acc[mt][:, :cw], wf2[:, mt * 128:(mt + 1) * 128],
                               g[:, :cw], start=(gmt == 0), stop=(gmt == 23))
                    for mt in range(CT):
                        nc.vector.scalar_tensor_tensor(
                            x[:, mt, co:co + cw], fc2acc[mt][:, :cw],
                            bt[:, 24 + mt:25 + mt], x[:, mt, co:co + cw],
                            op0=OP.add, op1=OP.add)

            # -------------- final LN + affine + transpose + out --------------
            xsq = bigpool.tile([128, CT, T], F32R, tag="big")
            act(xsq[:], x[:], AF.Square)
            xfin = bigpool.tile([128, CT, T], F32, tag="big")
            _ln_apply(nc, ps, smpool, stpool, ones, invc, eps_t, x, xsq, xfin)
            for ct in range(CT):
                act(xfin[:, ct, :], xfin[:, ct, :], AF.Identity,
                    bias=finb[:, ct:ct + 1], scale=fing[:, ct:ct + 1])
            for t in range(7):
                tw = min(128, T - t * 128)
                ob = obpool.tile([128, DIM], F32, tag="ob")
                for ct in range(CT):
                    ptr = ps.tile([128, 128], F32, tag="ps")
                    nc.tensor.transpose(ptr[:tw, :],
                                        xfin[:, ct, t * 128:t * 128 + tw],
                                        ident[:])
                    act(ob[:tw, ct * 128:(ct + 1) * 128], ptr[:tw, :], AF.Copy)
                nc.sync.dma_start(d_out[t * 128:t * 128 + tw, :], ob[:tw, :])

    nc.compile()
    return nc


_CACHED = {}


def _prep_host(inputs):
    """Host-side sharding + layout prep (numpy only)."""
    f = np.float32
    x = inputs["x"]
    qkv_w = np.asarray(inputs["qkv_w"], f)
    ln1_g, ln1_b = np.asarray(inputs["ln1_g"], f), np.asarray(inputs["ln1_b"], f)
    ln2_g, ln2_b = np.asarray(inputs["ln2_g"], f), np.asarray(inputs["ln2_b"], f)
    fc1_w, fc1_b = np.asarray(inputs["fc1_w"], f), np.asarray(inputs["fc1_b"], f)
    fc2_w, fc2_b = np.asarray(inputs["fc2_w"], f), np.asarray(inputs["fc2_b"], f)
    proj_w, proj_b = np.asarray(inputs["proj_w"], f), np.asarray(inputs["proj_b"], f)

    shared = {}
    # patch conv as GEMM: lhsT [cpq, o] split into 6 mt-slices [768, 128]
    pwT = np.asarray(inputs["patch_w"], f).reshape(DIM, 768).T     # [cpq, o]
    shared["patch_wT"] = np.ascontiguousarray(
        pwT.reshape(DIM, CT, 128).transpose(1, 0, 2))
    # qkv weights with ln1 affine fold, mt-sliced
    qkvw = ln1_g[:, :, None] * qkv_w                               # [12,768,2304]
    shared["qkv_wp"] = np.ascontiguousarray(
        qkvw.reshape(12, DIM, 18, 128)[:DEPTH].transpose(0, 2, 1, 3))
    attn_bias = np.einsum("dc,dco->do", ln1_b, qkv_w)              # [12, 2304]
    f1be = fc1_b + np.einsum("dc,dco->do", ln2_b, fc1_w)
    biasT = np.concatenate([
        attn_bias.reshape(12, 18, 128).transpose(0, 2, 1),
        proj_b.reshape(12, 6, 128).transpose(0, 2, 1),
        fc2_b.reshape(12, 6, 128).transpose(0, 2, 1),
        f1be.reshape(12, 24, 128).transpose(0, 2, 1),
    ], axis=2)                                                     # [12, 128, 54]
    shared["biasT"] = np.ascontiguousarray(biasT[:DEPTH]).astype(f)
    shared["proj_wp"] = np.ascontiguousarray(
        proj_w.reshape(12, DIM, CT, 128)[:DEPTH].transpose(0, 2, 1, 3))
    f1w = ln2_g[:, :, None] * fc1_w
    shared["fc1_wp"] = np.ascontiguousarray(
        f1w.reshape(12, DIM, 24, 128)[:DEPTH].transpose(0, 2, 1, 3))
    shared["fc2_w"] = np.ascontiguousarray(fc2_w[:DEPTH])
    shared["ones_c"] = np.ones((128, 512), f)
    shared["invc_c"] = np.full((1, 128), 1.0 / DIM, f)
    shared["zpad"] = np.zeros((128, 12 * (W - T)), f)
    shared["posc"] = np.ascontiguousarray(
        (np.asarray(inputs["pos_embed"], f)[0, 1:197]
         + np.asarray(inputs["patch_b"], f)[None, :]).T)
    extra = np.stack([
        np.asarray(inputs["cls_tok"], f)[0, 0] + np.asarray(inputs["pos_embed"], f)[0, 0],
        np.asarray(inputs["loc_tok"], f)[0, 0] + np.asarray(inputs["loc_embed"], f)[0, 0],
        np.asarray(inputs["loc_aug_tok"], f)[0, 0]
        + np.asarray(inputs["loc_aug_embed"], f)[0, 0],
    ], axis=1)
    shared["extra_cols"] = np.ascontiguousarray(extra).astype(f)
    shared["final_g"] = np.ascontiguousarray(
        np.asarray(inputs["norm_g"], f).reshape(CT, 128).T)
    shared["final_b"] = np.ascontiguousarray(
        np.asarray(inputs["norm_b"], f).reshape(CT, 128).T)

    in_maps = []
    for c in range(N_CORES):
        xs = np.asarray(x[BL * c:BL * (c + 1)], f)
        xT = np.ascontiguousarray(
            xs.reshape(BL, 3, 14, 16, 14, 16).transpose(1, 3, 5, 0, 2, 4)
            .reshape(DIM, BL * 196))
        m = dict(shared)
        m["xT"] = xT
        in_maps.append(m)
    return in_maps


def kernel(**inputs):
    if "nc" not in _CACHED:
        _CACHED["nc"] = build_program()
    nc = _CACHED["nc"]
    in_maps = _prep_host(inputs)
    res = run_bass_kernel_spmd(nc, in_maps, list(range(N_CORES)))
    out = np.concatenate([r["out"].reshape(BL, NTOK, DIM) for r in res.results],
                         axis=0)
    return out.astype(np.float32)



# revision 9
# speedup vs baseline: 1.1823x; 1.1823x over previous
"""ViT-style dense transformer (12 blocks, dim 768, 199 tokens, B=32) on 8
Trainium2 NeuronCores.

Sharding: data-parallel over batch — 4 images per core, no collectives.

Device layout: activations are kept channels-major (x.T, shape [768, tokens])
so every GEMM uses the weight as the stationary operand directly. The
residual stream x stays f32r; all GEMM operands (weights, LN outputs,
attention internals, gelu outputs) are bf16: halves the weight-DMA bytes,
runs matmuls at 1 cycle/row for any moving-dim size (so attention needs no
query padding), and halves DVE element time on the 16-bit tensors.
LayerNorm statistics (partition-dim sums) are ones-vector matmuls on the PE;
rstd is exp(-0.5*ln(var+eps)) so only the natural_log_exp + gelu ACT table
sets are used (no sqrt/sigmoid set reloads; the loc-mask sigmoid is computed
from exp). Softmax runs over the partition dim (keys) so the loc-mask is a
per-partition broadcast. LayerNorm affine params are folded into the
following weight matrices host-side; biases enter via activation bias.
"""
import contextlib
import os
import sys

sys.path.insert(0, "/opt/trn_rl_repo")

import ml_dtypes
import numpy as np

import concourse.bass as bass
import concourse.tile as tile
from concourse import bacc, mybir
from concourse.bass_utils import run_bass_kernel_spmd
from concourse.masks import make_identity

F32 = mybir.dt.float32
F32R = mybir.dt.float32r
BF16 = mybir.dt.bfloat16
AF = mybir.ActivationFunctionType
OP = mybir.AluOpType
NPBF = ml_dtypes.bfloat16

N_CORES = 8
BL = 4            # samples per core
DEPTH = int(os.environ.get("KDEPTH", "12"))
MASK_START = int(os.environ.get("KMASKSTART", "9"))
HEADS, DIM, HD = 12, 768, 64
SCALE = HD ** -0.5
NTOK = 199        # tokens per sample
T = BL * NTOK     # 796 tokens per core
CT = DIM // 128   # 6 channel tiles
CH = (398, 398)   # token chunks for N<=512 matmuls
FCH = (512, 284)  # token chunks for the fused MLP (fc2 psum = 6 banks + fc1 2)
EPS = 1e-5


def chunk_off(c):
    return sum(CH[:c])


def _ln_stats(nc, ps, smpool, stpool, ones1, invc, eps_t, x, xsq, c):
    """Per-token mean and rstd (both PSUM [128, cw] broadcasts) over the
    channel (partition x ct) axis of channels-major x, for token chunk c."""
    mm = nc.tensor.matmul
    act = nc.scalar.activation
    tt = nc.vector.tensor_tensor
    co, cw = chunk_off(c), CH[c]
    sraw = ps.tile([1, 398], F32, tag="ps")
    ssraw = ps.tile([1, 398], F32, tag="ps")
    for ct in range(CT):
        mm(sraw[:, :cw], ones1[:, 0:1], x[:, ct, co:co + cw],
           start=(ct == 0), stop=(ct == CT - 1))
        mm(ssraw[:, :cw], ones1[:, 0:1], xsq[:, ct, co:co + cw],
           start=(ct == 0), stop=(ct == CT - 1))
    srow = smpool.tile([1, 2, 398], F32R, tag="srow")
    act(srow[:, 0, :cw], sraw[:, :cw], AF.Copy)
    act(srow[:, 1, :cw], ssraw[:, :cw], AF.Copy)
    mu = ps.tile([128, 398], F32, tag="ps")
    msq = ps.tile([128, 398], F32, tag="ps")
    mm(mu[:, :cw], invc[:], srow[:, 0, :cw], start=True, stop=True)
    mm(msq[:, :cw], invc[:], srow[:, 1, :cw], start=True, stop=True)
    musq = stpool.tile([128, 398], F32, tag="lnsc")
    act(musq[:, :cw], mu[:, :cw], AF.Square)
    var = stpool.tile([128, 398], F32, tag="lnsc")
    tt(var[:, :cw], msq[:, :cw], musq[:, :cw], op=OP.subtract)
    # rstd = exp(-0.5 * ln(var + eps)): stays in the natural_log_exp ACT
    # table set (no sqrt-set reload)
    lv = stpool.tile([128, 398], F32, tag="lnsc")
    act(lv[:, :cw], var[:, :cw], AF.Ln, bias=eps_t[:, 0:1])
    rstd = ps.tile([128, 398], F32, tag="ps")
    act(rstd[:, :cw], lv[:, :cw], AF.Exp, scale=-0.5)
    return mu, rstd


def _ln_apply(nc, ps, smpool, stpool, ones1, invc, eps_t, x, xsq, h):
    """h = (x - mu) * rstd, channels-major, chunk at a time (h may be bf16)."""
    tt = nc.vector.tensor_tensor
    for c in range(2):
        co, cw = chunk_off(c), CH[c]
        mu, rstd = _ln_stats(nc, ps, smpool, stpool, ones1, invc, eps_t,
                             x, xsq, c)
        for ct in range(CT):
            tt(h[:, ct, co:co + cw], x[:, ct, co:co + cw], mu[:, :cw],
               op=OP.subtract)
            tt(h[:, ct, co:co + cw], h[:, ct, co:co + cw], rstd[:, :cw],
               op=OP.mult)


def build_program():
    nc = bacc.Bacc("TRN2", target_bir_lowering=False, debug=False,
                   num_devices=N_CORES)

    # ---- DRAM parameters (per-core views, host-prepped) ----
    d_xT = nc.dram_tensor("xT", [DIM, BL * 196], BF16, kind="ExternalInput")
    d_pw = nc.dram_tensor("patch_wT", [CT, DIM, 128], BF16, kind="ExternalInput")
    d_qkvw = nc.dram_tensor("qkv_wp", [DEPTH, 18, DIM, 128], BF16,
                            kind="ExternalInput")
    d_bias = nc.dram_tensor("biasT", [DEPTH, 128, 54], F32, kind="ExternalInput")
    d_prw = nc.dram_tensor("proj_wp", [DEPTH, CT, DIM, 128], BF16,
                           kind="ExternalInput")
    d_f1w = nc.dram_tensor("fc1_wp", [DEPTH, 24, DIM, 128], BF16,
                           kind="ExternalInput")
    d_f2w = nc.dram_tensor("fc2_w", [DEPTH, 4 * DIM, DIM], BF16,
                           kind="ExternalInput")
    d_posc = nc.dram_tensor("posc", [DIM, 196], F32, kind="ExternalInput")
    d_extra = nc.dram_tensor("extra_cols", [DIM, 3], F32, kind="ExternalInput")
    d_fing = nc.dram_tensor("final_g", [128, CT], F32, kind="ExternalInput")
    d_finb = nc.dram_tensor("final_b", [128, CT], F32, kind="ExternalInput")
    d_out = nc.dram_tensor("out", [T, DIM], F32, kind="ExternalOutput")

    mm = nc.tensor.matmul
    act = nc.scalar.activation
    tt = nc.vector.tensor_tensor
    ts = nc.vector.tensor_scalar

    KREPEAT = int(os.environ.get("KREPEAT", "1"))
    with tile.TileContext(nc) as tc:
        rep = contextlib.ExitStack()
        if KREPEAT > 1:
            rep.enter_context(tc.For_i(0, KREPEAT, 1))
        with (
            rep,
            tc.tile_pool(name="const", bufs=1) as cpool,
            tc.tile_pool(name="x", bufs=1) as xpool,
            tc.tile_pool(name="big", bufs=2) as bigpool,    # xsq/h/vT/oT/h2 rotate
            tc.tile_pool(name="qk", bufs=1) as qkpool,
            tc.tile_pool(name="v", bufs=1) as vpool,
            tc.tile_pool(name="aa", bufs=1) as aapool,
            tc.tile_pool(name="rr", bufs=1) as rrpool,
            tc.tile_pool(name="wmt", bufs=6) as wmtpool,    # [128,6,128] mt-slices
            tc.tile_pool(name="wkt", bufs=4) as wktpool,    # [128,768] k-slices
            tc.tile_pool(name="bias", bufs=1) as biaspool,
            tc.tile_pool(name="gelu", bufs=3) as gelupool,
            tc.tile_pool(name="stats", bufs=2) as stpool,
            tc.tile_pool(name="small", bufs=1) as smpool,
            tc.tile_pool(name="obuf", bufs=2) as obpool,
            tc.tile_pool(name="ps", bufs=8, space="PSUM") as ps,
        ):
            # ---------------- constants ----------------
            onesf = cpool.tile([128, 128], F32, tag="onesf")
            nc.vector.memset(onesf[:], 1.0)
            onesb = cpool.tile([128, 128], BF16, tag="onesb")
            act(onesb[:], onesf[:], AF.Copy)
            ones1 = cpool.tile([128, 1], F32R, tag="ones1")
            act(ones1[:], onesf[:, 0:1], AF.Copy)
            invc = cpool.tile([1, 128], F32R, tag="invc")
            act(invc[:], onesf[0:1, :], AF.Copy, scale=1.0 / DIM)
            eps_t = cpool.tile([128, 1], F32, tag="eps")
            nc.vector.memset(eps_t[:], EPS)
            ident = cpool.tile([128, 128], F32, tag="ident")
            make_identity(nc, ident[:])
            identb = cpool.tile([128, 128], BF16, tag="identb")
            act(identb[:], ident[:], AF.Copy)
            fing = cpool.tile([128, CT], F32, tag="fing")
            nc.sync.dma_start(fing[:], d_fing[:])
            finb = cpool.tile([128, CT], F32, tag="finb")
            nc.sync.dma_start(finb[:], d_finb[:])

            # residual stream, channels-major: x[p, ct, tok]
            x = xpool.tile([128, CT, T], F32R, tag="x")

            # ---------------- patch embed ----------------
            with tc.tile_pool(name="patch", bufs=1) as ppool:
                posc = ppool.tile([128, CT, 196], F32, tag="posc")
                nc.sync.dma_start(posc[:],
                                  d_posc.rearrange("(ct p) t -> p ct t", p=128))
                extra = ppool.tile([128, CT, 3], F32, tag="extra")
                nc.sync.dma_start(extra[:],
                                  d_extra.rearrange("(ct p) t -> p ct t", p=128))
                xt = ppool.tile([128, CT, BL * 196], BF16, tag="xt")
                nc.sync.dma_start(xt[:], d_xT.rearrange("(kt p) t -> p kt t", p=128))
                for mt in range(CT):
                    pw = wmtpool.tile([128, CT, 128], BF16, tag="wmt")
                    nc.sync.dma_start(
                        pw[:], d_pw[mt].rearrange("(kt p) m -> p kt m", p=128))
                    for c in range(2):  # 392-token halves: samples (2c, 2c+1)
                        acc = ps.tile([128, 392], F32, tag="ps")
                        for kt in range(CT):
                            mm(acc[:], pw[:, kt, :],
                               xt[:, kt, c * 392:(c + 1) * 392],
                               start=(kt == 0), stop=(kt == CT - 1))
                        for si in range(2):
                            s = 2 * c + si
                            tt(x[:, mt, s * NTOK + 1: s * NTOK + 197],
                               acc[:, si * 196:(si + 1) * 196], posc[:, mt, :],
                               op=OP.add)
                for s in range(BL):
                    o = s * NTOK
                    nc.vector.tensor_copy(x[:, :, o:o + 1], extra[:, :, 0:1])
                    nc.vector.tensor_copy(x[:, :, o + 197:o + 199],
                                          extra[:, :, 1:3])

            # ---------------- transformer blocks ----------------
            for i in range(DEPTH):
                # ===== LN1 -> h (affine folded into qkv weights) =====
                xsq = bigpool.tile([128, CT, T], F32R, tag="big")
                act(xsq[:], x[:], AF.Square)
                h = bigpool.tile([128, CT, T], BF16, tag="big")
                _ln_apply(nc, ps, smpool, stpool, ones1, invc, eps_t, x, xsq, h)

                # ===== qkv = h @ Wqkv (channels-major out) =====
                qk = qkpool.tile([128, 12, T], BF16, tag="qk")
                vT = bigpool.tile([128, CT, T], BF16, tag="big")
                bt = biaspool.tile([128, 54], F32, tag="bt")
                nc.sync.dma_start(bt[:], d_bias[i])
                for mt in range(18):
                    wq = wmtpool.tile([128, CT, 128], BF16, tag="wmt")
                    nc.sync.dma_start(
                        wq[:], d_qkvw[i, mt].rearrange("(kt p) m -> p kt m", p=128))
                    for c in range(2):
                        co, cw = chunk_off(c), CH[c]
                        acc = ps.tile([128, 398], F32, tag="ps")
                        for kt in range(CT):
                            mm(acc[:, :cw], wq[:, kt, :], h[:, kt, co:co + cw],
                               start=(kt == 0), stop=(kt == CT - 1))
                        dst = (qk[:, mt, co:co + cw] if mt < 12
                               else vT[:, mt - 12, co:co + cw])
                        act(dst, acc[:, :cw], AF.Identity, bias=bt[:, mt:mt + 1])

                # ===== transpose v to tokens-major v0/v1 =====
                v0 = vpool.tile([128, BL, DIM], BF16, tag="v0")
                v1 = vpool.tile([128, BL, DIM], BF16, tag="v1")
                for s in range(BL):
                    o = s * NTOK
                    for cc, (vt, mw) in enumerate(((v0, 128), (v1, 71))):
                        for ct in range(CT):
                            ptr = ps.tile([128, 128], BF16, tag="ps")
                            nc.tensor.transpose(
                                ptr[:mw, :],
                                vT[:, ct, o + cc * 128: o + cc * 128 + mw],
                                identb[:])
                            act(vt[:mw, s, ct * 128:(ct + 1) * 128],
                                ptr[:mw, :], AF.Copy)

                # ===== attention =====
                oT = bigpool.tile([128, CT, T], BF16, tag="big")
                use_mask = i >= MASK_START
                for s in range(BL):
                    o = s * NTOK
                    aa = aapool.tile([128, 12, NTOK], BF16, tag="aa")
                    ab2 = aapool.tile([128, 12, NTOK], BF16, tag="ab2")
                    if use_mask:
                        macc0 = smpool.tile([128, 2], F32, tag="macc0")
                        macc1 = smpool.tile([128, 2], F32, tag="macc1")
                    for h_ in range(12):
                        ro = (h_ % 2) * 64
                        jq, jk = h_ // 2, 6 + h_ // 2
                        ssum = ps.tile([1, NTOK], F32, tag="ps")
                        for cc, mw in ((0, 128), (1, 71)):
                            S = ps.tile([128, NTOK], F32, tag="ps")
                            mm(S[:mw, :],
                               qk[ro:ro + 64, jk, o + cc * 128: o + cc * 128 + mw],
                               qk[ro:ro + 64, jq, o:o + NTOK],
                               start=True, stop=True)
                            at = (aa, ab2)[cc]
                            act(at[:mw, h_, :], S[:mw, :], AF.Exp, scale=SCALE)
                            if use_mask:
                                macc = (macc0, macc1)[cc]
                                if h_ == 0:
                                    nc.vector.tensor_copy(macc[:mw, :],
                                                          S[:mw, 197:199])
                                else:
                                    tt(macc[:mw, :], macc[:mw, :],
                                       S[:mw, 197:199], op=OP.add)
                            mm(ssum[:, :], onesb[:mw, 0:1], at[:mw, h_, :],
                               start=(cc == 0), stop=(cc == 1))
                        rr = rrpool.tile([1, NTOK], BF16, tag="rr")
                        with nc.allow_low_precision(reason="softmax denom bf16"):
                            nc.vector.reciprocal(rr[:], ssum[:])
                        rb = ps.tile([128, NTOK], F32, tag="ps")
                        mm(rb[:], onesb[0:1, 0:128], rr[:], start=True, stop=True)
                        tt(aa[:, h_, :], aa[:, h_, :], rb[:128, :], op=OP.mult)
                        tt(ab2[:71, h_, :], ab2[:71, h_, :], rb[:71, :],
                           op=OP.mult)
                    if use_mask:
                        for cc, mw in ((0, 128), (1, 71)):
                            macc = (macc0, macc1)[cc]
                            # sigmoid(z) = 1/(1+exp(-z)) — avoids the
                            # sigmoid ACT table set
                            sge = smpool.tile([128, 2], F32, tag="sge")
                            act(sge[:mw, :], macc[:mw, :], AF.Exp,
                                scale=-SCALE / 12)
                            sgd = smpool.tile([128, 2], F32, tag="sgd")
                            ts(sgd[:mw, :], sge[:mw, :], 1.0, None, op0=OP.add)
                            sg = smpool.tile([128, 2], F32, tag="sg")
                            nc.vector.reciprocal(sg[:mw, :], sgd[:mw, :])
                            mk = smpool.tile([128, 1], F32, tag="mk")
                            tt(mk[:mw, :], sg[:mw, 0:1], sg[:mw, 1:2], op=OP.max)
                            at = (aa, ab2)[cc]
                            for h_ in range(12):
                                ts(at[:mw, h_, :], at[:mw, h_, :], mk[:mw, 0:1],
                                   None, op0=OP.mult)
                    for h_ in range(12):
                        ro, j = (h_ % 2) * 64, h_ // 2
                        Oh = ps.tile([64, NTOK], F32, tag="ps")
                        mm(Oh[:, :], v0[:, s, h_ * 64:(h_ + 1) * 64],
                           aa[:, h_, :], start=True, stop=False)
                        mm(Oh[:, :], v1[:71, s, h_ * 64:(h_ + 1) * 64],
                           ab2[:71, h_, :], start=False, stop=True)
                        if ro == 0:
                            act(oT[0:64, j, o:o + NTOK], Oh[:, :NTOK], AF.Copy)
                        else:
                            stg = smpool.tile([64, NTOK], BF16, tag="stg")
                            act(stg[:, :], Oh[:, :NTOK], AF.Copy)
                            nc.sync.dma_start(oT[64:128, j, o:o + NTOK], stg[:, :])

                # ===== proj + residual =====
                for mt in range(CT):
                    wp = wmtpool.tile([128, CT, 128], BF16, tag="wmt")
                    nc.sync.dma_start(
                        wp[:], d_prw[i, mt].rearrange("(kt p) m -> p kt m", p=128))
                    for c in range(2):
                        co, cw = chunk_off(c), CH[c]
                        acc = ps.tile([128, 398], F32, tag="ps")
                        for kt in range(CT):
                            mm(acc[:, :cw], wp[:, kt, :], oT[:, kt, co:co + cw],
                               start=(kt == 0), stop=(kt == CT - 1))
                        nc.vector.scalar_tensor_tensor(
                            x[:, mt, co:co + cw], acc[:, :cw],
                            bt[:, 18 + mt:19 + mt], x[:, mt, co:co + cw],
                            op0=OP.add, op1=OP.add)

                # ===== LN2 -> h2 =====
                xsq = bigpool.tile([128, CT, T], F32R, tag="big")
                act(xsq[:], x[:], AF.Square)
                h2 = bigpool.tile([128, CT, T], BF16, tag="big")
                _ln_apply(nc, ps, smpool, stpool, ones1, invc, eps_t, x, xsq, h2)

                # ===== MLP: fc1 -> gelu -> fc2 (streamed, fc2 psum resident) ====
                for c in range(2):
                    co = sum(FCH[:c])
                    cw = FCH[c]
                    fc2acc = []
                    for mt in range(CT):
                        a2 = ps.tile([128, 512], F32, tag="ps")
                        fc2acc.append(a2)
                    for gmt in range(24):
                        wf = wmtpool.tile([128, CT, 128], BF16, tag="wmt")
                        nc.sync.dma_start(
                            wf[:], d_f1w[i, gmt].rearrange("(kt p) m -> p kt m",
                                                           p=128))
                        a1 = ps.tile([128, 512], F32, tag="ps")
                        for kt in range(CT):
                            mm(a1[:, :cw], wf[:, kt, :], h2[:, kt, co:co + cw],
                               start=(kt == 0), stop=(kt == CT - 1))
                        g = gelupool.tile([128, 512], BF16, tag="gelu")
                        act(g[:, :cw], a1[:, :cw], AF.Gelu,
                            bias=bt[:, 30 + gmt:31 + gmt])
                        wf2 = wktpool.tile([128, DIM], BF16, tag="wkt")
                        nc.sync.dma_start(
                            wf2[:], d_f2w[i, gmt * 128:(gmt + 1) * 128, :])
                        for mt in range(CT):
                            mm(fc2acc[mt][:, :cw], wf2[:, mt * 128:(mt + 1) * 128],
                               g[:, :cw], start=(gmt == 0), stop=(gmt == 23))
                    for mt in range(CT):
                        nc.vector.scalar_tensor_tensor(
                            x[:, mt, co:co + cw], fc2acc[mt][:, :cw],
                            bt[:, 24 + mt:25 + mt], x[:, mt, co:co + cw],
                            op0=OP.add, op1=OP.add)

            # -------------- final LN + affine + transpose + out --------------
            xsq = bigpool.tile([128, CT, T], F32R, tag="big")
            act(xsq[:], x[:], AF.Square)
            xfin = bigpool.tile([128, CT, T], F32, tag="big")
            _ln_apply(nc, ps, smpool, stpool, ones1, invc, eps_t, x, xsq, xfin)
            for ct in range(CT):
                act(xfin[:, ct, :], xfin[:, ct, :], AF.Identity,
                    bias=finb[:, ct:ct + 1], scale=fing[:, ct:ct + 1])
            for t in range(7):
                tw = min(128, T - t * 128)
                ob = obpool.tile([128, DIM], F32, tag="ob")
                for ct in range(CT):
                    ptr = ps.tile([128, 128], F32, tag="ps")
                    nc.tensor.transpose(ptr[:tw, :],
                                        xfin[:, ct, t * 128:t * 128 + tw],
                                        ident[:])
                    act(ob[:tw, ct * 128:(ct + 1) * 128], ptr[:tw, :], AF.Copy)
                nc.sync.dma_start(d_out[t * 128:t * 128 + tw, :], ob[:tw, :])

    nc.compile()
    return nc


_CACHED = {}


def _prep_host(inputs):
    """Host-side sharding + layout prep (numpy only)."""
    f = np.float32
    x = inputs["x"]
    qkv_w = np.asarray(inputs["qkv_w"], f)
    ln1_g, ln1_b = np.asarray(inputs["ln1_g"], f), np.asarray(inputs["ln1_b"], f)
    ln2_g, ln2_b = np.asarray(inputs["ln2_g"], f), np.asarray(inputs["ln2_b"], f)
    fc1_w, fc1_b = np.asarray(inputs["fc1_w"], f), np.asarray(inputs["fc1_b"], f)
    fc2_w, fc2_b = np.asarray(inputs["fc2_w"], f), np.asarray(inputs["fc2_b"], f)
    proj_w, proj_b = np.asarray(inputs["proj_w"], f), np.asarray(inputs["proj_b"], f)

    shared = {}
    # patch conv as GEMM: lhsT [cpq, o] split into 6 mt-slices [768, 128]
    pwT = np.asarray(inputs["patch_w"], f).reshape(DIM, 768).T     # [cpq, o]
    shared["patch_wT"] = np.ascontiguousarray(
        pwT.reshape(DIM, CT, 128).transpose(1, 0, 2)).astype(NPBF)
    # qkv weights with ln1 affine fold, mt-sliced
    qkvw = ln1_g[:, :, None] * qkv_w                               # [12,768,2304]
    shared["qkv_wp"] = np.ascontiguousarray(
        qkvw.reshape(12, DIM, 18, 128)[:DEPTH].transpose(0, 2, 1, 3)).astype(NPBF)
    attn_bias = np.einsum("dc,dco->do", ln1_b, qkv_w)              # [12, 2304]
    f1be = fc1_b + np.einsum("dc,dco->do", ln2_b, fc1_w)
    biasT = np.concatenate([
        attn_bias.reshape(12, 18, 128).transpose(0, 2, 1),
        proj_b.reshape(12, 6, 128).transpose(0, 2, 1),
        fc2_b.reshape(12, 6, 128).transpose(0, 2, 1),
        f1be.reshape(12, 24, 128).transpose(0, 2, 1),
    ], axis=2)                                                     # [12, 128, 54]
    shared["biasT"] = np.ascontiguousarray(biasT[:DEPTH]).astype(f)
    shared["proj_wp"] = np.ascontiguousarray(
        proj_w.reshape(12, DIM, CT, 128)[:DEPTH].transpose(0, 2, 1, 3)).astype(NPBF)
    f1w = ln2_g[:, :, None] * fc1_w
    shared["fc1_wp"] = np.ascontiguousarray(
        f1w.reshape(12, DIM, 24, 128)[:DEPTH].transpose(0, 2, 1, 3)).astype(NPBF)
    shared["fc2_w"] = np.ascontiguousarray(fc2_w[:DEPTH]).astype(NPBF)
    shared["posc"] = np.ascontiguousarray(
        (np.asarray(inputs["pos_embed"], f)[0, 1:197]
         + np.asarray(inputs["patch_b"], f)[None, :]).T)
    extra = np.stack([
        np.asarray(inputs["cls_tok"], f)[0, 0] + np.asarray(inputs["pos_embed"], f)[0, 0],
        np.asarray(inputs["loc_tok"], f)[0, 0] + np.asarray(inputs["loc_embed"], f)[0, 0],
        np.asarray(inputs["loc_aug_tok"], f)[0, 0]
        + np.asarray(inputs["loc_aug_embed"], f)[0, 0],
    ], axis=1)
    shared["extra_cols"] = np.ascontiguousarray(extra).astype(f)
    shared["final_g"] = np.ascontiguousarray(
        np.asarray(inputs["norm_g"], f).reshape(CT, 128).T)
    shared["final_b"] = np.ascontiguousarray(
        np.asarray(inputs["norm_b"], f).reshape(CT, 128).T)

    in_maps = []
    for c in range(N_CORES):
        xs = np.asarray(x[BL * c:BL * (c + 1)], f)
        xT = np.ascontiguousarray(
            xs.reshape(BL, 3, 14, 16, 14, 16).transpose(1, 3, 5, 0, 2, 4)
            .reshape(DIM, BL * 196))
        m = dict(shared)
        m["xT"] = xT.astype(NPBF)
        in_maps.append(m)
    return in_maps


def kernel(**inputs):
    if "nc" not in _CACHED:
        _CACHED["nc"] = build_program()
    nc = _CACHED["nc"]
    in_maps = _prep_host(inputs)
    res = run_bass_kernel_spmd(nc, in_maps, list(range(N_CORES)))
    out = np.concatenate([r["out"].reshape(BL, NTOK, DIM) for r in res.results],
                         axis=0)
    return out.astype(np.float32)


# revision 14
# speedup vs baseline: 1.4780x; 1.2500x over previous
"""ViT-style dense transformer (12 blocks, dim 768, 199 tokens, B=32) on 8
Trainium2 NeuronCores.

Sharding: data-parallel over batch — 4 images per core, no collectives.

Device layout: activations are kept channels-major (x.T, shape [768, tokens])
so every GEMM uses the weight as the stationary operand directly. The
residual stream x stays f32r; all GEMM operands (weights, LN outputs,
attention internals, gelu outputs) are bf16: halves the weight-DMA bytes,
runs matmuls at 1 cycle/row for any moving-dim size (so attention needs no
query padding), and halves DVE element time on the 16-bit tensors.
LayerNorm statistics (partition-dim sums) are ones-vector matmuls on the PE;
rstd is exp(-0.5*ln(var+eps)) so only the natural_log_exp + gelu ACT table
sets are used (no sqrt/sigmoid set reloads; the loc-mask sigmoid is computed
from exp). Softmax runs over the partition dim (keys) so the loc-mask is a
per-partition broadcast. LayerNorm affine params are folded into the
following weight matrices host-side; biases enter via activation bias.
"""
import contextlib
import os
import sys

sys.path.insert(0, "/opt/trn_rl_repo")

import ml_dtypes
import numpy as np

import concourse.bass as bass
import concourse.tile as tile
from concourse import bacc, mybir
from concourse.bass_utils import run_bass_kernel_spmd
from concourse.masks import make_identity

F32 = mybir.dt.float32
F32R = mybir.dt.float32r
BF16 = mybir.dt.bfloat16
AF = mybir.ActivationFunctionType
OP = mybir.AluOpType
NPBF = ml_dtypes.bfloat16

N_CORES = 8
BL = 4            # samples per core
DEPTH = int(os.environ.get("KDEPTH", "12"))
MASK_START = int(os.environ.get("KMASKSTART", "9"))
HEADS, DIM, HD = 12, 768, 64
SCALE = HD ** -0.5
NTOK = 199        # tokens per sample
T = BL * NTOK     # 796 tokens per core
CT = DIM // 128   # 6 channel tiles
CH = (398, 398)   # token chunks for N<=512 matmuls
FCH = (512, 284)  # token chunks for the fused MLP (fc2 psum = 6 banks + fc1 2)
EPS = 1e-5


def chunk_off(c):
    return sum(CH[:c])


def _ln_stats(nc, ps, smpool, stpool, ones1, invc, eps_t, x, xsq, c):
    """Per-token mean and rstd (both PSUM [128, cw] broadcasts) over the
    channel (partition x ct) axis of channels-major x, for token chunk c."""
    mm = nc.tensor.matmul
    act = nc.scalar.activation
    tt = nc.vector.tensor_tensor
    co, cw = chunk_off(c), CH[c]
    sraw = ps.tile([1, 398], F32, tag="ps")
    ssraw = ps.tile([1, 398], F32, tag="ps")
    for ct in range(CT):
        mm(sraw[:, :cw], ones1[:, 0:1], x[:, ct, co:co + cw],
           start=(ct == 0), stop=(ct == CT - 1))
        mm(ssraw[:, :cw], ones1[:, 0:1], xsq[:, ct, co:co + cw],
           start=(ct == 0), stop=(ct == CT - 1))
    srow = smpool.tile([1, 2, 398], F32R, tag="srow")
    act(srow[:, 0, :cw], sraw[:, :cw], AF.Copy)
    act(srow[:, 1, :cw], ssraw[:, :cw], AF.Copy)
    mu = ps.tile([128, 398], F32, tag="ps")
    msq = ps.tile([128, 398], F32, tag="ps")
    mm(mu[:, :cw], invc[:], srow[:, 0, :cw], start=True, stop=True)
    mm(msq[:, :cw], invc[:], srow[:, 1, :cw], start=True, stop=True)
    musq = stpool.tile([128, 398], F32, tag="lnsc")
    act(musq[:, :cw], mu[:, :cw], AF.Square)
    var = stpool.tile([128, 398], F32, tag="lnsc")
    tt(var[:, :cw], msq[:, :cw], musq[:, :cw], op=OP.subtract)
    # rstd = exp(-0.5 * ln(var + eps)): stays in the natural_log_exp ACT
    # table set (no sqrt-set reload)
    lv = stpool.tile([128, 398], F32, tag="lnsc")
    act(lv[:, :cw], var[:, :cw], AF.Ln, bias=eps_t[:, 0:1])
    rstd = ps.tile([128, 398], F32, tag="ps")
    act(rstd[:, :cw], lv[:, :cw], AF.Exp, scale=-0.5)
    return mu, rstd


def _ln_apply(nc, ps, smpool, stpool, ones1, invc, eps_t, x, xsq, h):
    """h = (x - mu) * rstd, channels-major, chunk at a time (h may be bf16)."""
    tt = nc.vector.tensor_tensor
    for c in range(2):
        co, cw = chunk_off(c), CH[c]
        mu, rstd = _ln_stats(nc, ps, smpool, stpool, ones1, invc, eps_t,
                             x, xsq, c)
        for ct in range(CT):
            tt(h[:, ct, co:co + cw], x[:, ct, co:co + cw], mu[:, :cw],
               op=OP.subtract)
            tt(h[:, ct, co:co + cw], h[:, ct, co:co + cw], rstd[:, :cw],
               op=OP.mult)


def _attn_sample(nc, ps, aapool, rrpool, smpool, qk, v0, v1, oT, onesb, s,
                 use_mask, first_mask_head):
    """Attention for one sample: scores/softmax/AV, head-pair interleaved.

    aa holds both key chunks per head: cc0 (128 keys) at cols 0:199, cc1
    (71 keys) at cols 256:455. Scores for a head land in ONE psum bank
    (S2 [128,512]); head pairs (rows 0:64 / 64:128 of qk) interleave so
    the PE can overlap their matmuls across row groups, and AV packs each
    pair into one [128,199] psum via col tiling so the copy to
    channels-major oT is partition-aligned (no DMA staging).
    """
    mm = nc.tensor.matmul
    act = nc.scalar.activation
    tt = nc.vector.tensor_tensor
    ts = nc.vector.tensor_scalar
    CO1 = 256
    o = s * NTOK
    aa = aapool.tile([128, 12, 512], BF16, tag="aa")
    macc0 = macc1 = None
    if use_mask:
        macc0 = smpool.tile([128, 2], F32, tag="macc0")
        macc1 = smpool.tile([128, 2], F32, tag="macc1")
    for hp in range(6):
        jq, jk = hp, 6 + hp
        S2s = []
        for idx in range(2):
            ro = idx * 64
            S2 = ps.tile([128, 512], F32, tag="ps")
            S2s.append(S2)
            for cc, mw in ((0, 128), (1, 71)):
                mm(S2[:mw, cc * CO1:cc * CO1 + NTOK],
                   qk[ro:ro + 64, jk, o + cc * 128: o + cc * 128 + mw],
                   qk[ro:ro + 64, jq, o:o + NTOK],
                   start=True, stop=True)
        for idx in range(2):
            h_ = 2 * hp + idx
            S2 = S2s[idx]
            for cc, mw in ((0, 128), (1, 71)):
                act(aa[:mw, h_, cc * CO1:cc * CO1 + NTOK],
                    S2[:mw, cc * CO1:cc * CO1 + NTOK], AF.Exp, scale=SCALE)
                if use_mask:
                    macc = (macc0, macc1)[cc]
                    sl = S2[:mw, cc * CO1 + 197:cc * CO1 + 199]
                    if h_ == 0:
                        nc.vector.tensor_copy(macc[:mw, :], sl)
                    else:
                        tt(macc[:mw, :], macc[:mw, :], sl, op=OP.add)
            ssum = ps.tile([1, NTOK], F32, tag="ps")
            mm(ssum[:, :], onesb[:128, 0:1], aa[:128, h_, 0:NTOK],
               start=True, stop=False)
            mm(ssum[:, :], onesb[:71, 0:1], aa[:71, h_, CO1:CO1 + NTOK],
               start=False, stop=True)
            rr = rrpool.tile([1, NTOK], BF16, tag="rr")
            with nc.allow_low_precision(reason="softmax bf16"):
                nc.vector.reciprocal(rr[:], ssum[:])
            rb = ps.tile([128, NTOK], F32, tag="ps")
            mm(rb[:], onesb[0:1, 0:128], rr[:], start=True, stop=True)
            tt(aa[:, h_, 0:NTOK], aa[:, h_, 0:NTOK], rb[:128, :], op=OP.mult)
            tt(aa[:71, h_, CO1:CO1 + NTOK], aa[:71, h_, CO1:CO1 + NTOK],
               rb[:71, :], op=OP.mult)
    if use_mask:
        for cc, mw in ((0, 128), (1, 71)):
            macc = (macc0, macc1)[cc]
            # sigmoid(z) = 1/(1+exp(-z)) — avoids the sigmoid ACT table set
            sge = smpool.tile([128, 2], F32, tag="sge")
            act(sge[:mw, :], macc[:mw, :], AF.Exp, scale=-SCALE / 12)
            sgd = smpool.tile([128, 2], F32, tag="sgd")
            ts(sgd[:mw, :], sge[:mw, :], 1.0, None, op0=OP.add)
            sg = smpool.tile([128, 2], F32, tag="sg")
            nc.vector.reciprocal(sg[:mw, :], sgd[:mw, :])
            mk = smpool.tile([128, 1], F32, tag="mk")
            tt(mk[:mw, :], sg[:mw, 0:1], sg[:mw, 1:2], op=OP.max)
            coff = cc * CO1
            for h_ in range(12):
                ts(aa[:mw, h_, coff:coff + NTOK],
                   aa[:mw, h_, coff:coff + NTOK], mk[:mw, 0:1], None,
                   op0=OP.mult)
    for hp in range(6):
        OhP = ps.tile([128, NTOK], F32, tag="ps")
        for idx in range(2):
            h_ = 2 * hp + idx
            bp = idx * 64
            mm(OhP[bp:bp + 64, :], v0[:, s, h_ * 64:(h_ + 1) * 64],
               aa[:, h_, 0:NTOK], start=True, stop=False,
               tile_position=(0, bp))
            mm(OhP[bp:bp + 64, :], v1[:71, s, h_ * 64:(h_ + 1) * 64],
               aa[:71, h_, CO1:CO1 + NTOK], start=False, stop=True,
               tile_position=(0, bp))
        act(oT[:, hp, o:o + NTOK], OhP[:, :], AF.Copy)


def build_program():
    nc = bacc.Bacc("TRN2", target_bir_lowering=False, debug=False,
                   num_devices=N_CORES)

    # ---- DRAM parameters (per-core views, host-prepped) ----
    d_xT = nc.dram_tensor("xT", [DIM, BL * 196], BF16, kind="ExternalInput")
    d_pw = nc.dram_tensor("patch_wT", [CT, DIM, 128], BF16, kind="ExternalInput")
    d_qkvw = nc.dram_tensor("qkv_wp", [DEPTH, 18, DIM, 128], BF16,
                            kind="ExternalInput")
    d_bias = nc.dram_tensor("biasT", [DEPTH, 128, 54], F32, kind="ExternalInput")
    d_prw = nc.dram_tensor("proj_wp", [DEPTH, CT, DIM, 128], BF16,
                           kind="ExternalInput")
    d_f1w = nc.dram_tensor("fc1_wp", [DEPTH, 24, DIM, 128], BF16,
                           kind="ExternalInput")
    d_f2w = nc.dram_tensor("fc2_w", [DEPTH, 4 * DIM, DIM], BF16,
                           kind="ExternalInput")
    d_posc = nc.dram_tensor("posc", [DIM, 196], F32, kind="ExternalInput")
    d_extra = nc.dram_tensor("extra_cols", [DIM, 3], F32, kind="ExternalInput")
    d_fing = nc.dram_tensor("final_g", [128, CT], F32, kind="ExternalInput")
    d_finb = nc.dram_tensor("final_b", [128, CT], F32, kind="ExternalInput")
    d_out = nc.dram_tensor("out", [T, DIM], F32, kind="ExternalOutput")

    mm = nc.tensor.matmul
    act = nc.scalar.activation
    tt = nc.vector.tensor_tensor
    ts = nc.vector.tensor_scalar

    KREPEAT = int(os.environ.get("KREPEAT", "1"))
    with tile.TileContext(nc) as tc:
        rep = contextlib.ExitStack()
        if KREPEAT > 1:
            rep.enter_context(tc.For_i(0, KREPEAT, 1))
        with (
            rep,
            tc.tile_pool(name="const", bufs=1) as cpool,
            tc.tile_pool(name="x", bufs=1) as xpool,
            tc.tile_pool(name="big", bufs=2) as bigpool,    # xsq/h/vT/oT/h2 rotate
            tc.tile_pool(name="qk", bufs=1) as qkpool,
            tc.tile_pool(name="v", bufs=1) as vpool,
            tc.tile_pool(name="aa", bufs=2) as aapool,
            tc.tile_pool(name="rr", bufs=2) as rrpool,
            tc.tile_pool(name="wmt", bufs=6) as wmtpool,    # [128,6,128] mt-slices
            tc.tile_pool(name="wkt", bufs=4) as wktpool,    # [128,768] k-slices
            tc.tile_pool(name="bias", bufs=1) as biaspool,
            tc.tile_pool(name="gelu", bufs=3) as gelupool,
            tc.tile_pool(name="stats", bufs=2) as stpool,
            tc.tile_pool(name="small", bufs=1) as smpool,
            tc.tile_pool(name="obuf", bufs=2) as obpool,
            tc.tile_pool(name="ps", bufs=8, space="PSUM") as ps,
        ):
            # ---------------- constants ----------------
            onesf = cpool.tile([128, 128], F32, tag="onesf")
            nc.vector.memset(onesf[:], 1.0)
            onesb = cpool.tile([128, 128], BF16, tag="onesb")
            act(onesb[:], onesf[:], AF.Copy)
            ones1 = cpool.tile([128, 1], F32R, tag="ones1")
            act(ones1[:], onesf[:, 0:1], AF.Copy)
            invc = cpool.tile([1, 128], F32R, tag="invc")
            act(invc[:], onesf[0:1, :], AF.Copy, scale=1.0 / DIM)
            eps_t = cpool.tile([128, 1], F32, tag="eps")
            nc.vector.memset(eps_t[:], EPS)
            ident = cpool.tile([128, 128], F32, tag="ident")
            make_identity(nc, ident[:])
            identb = cpool.tile([128, 128], BF16, tag="identb")
            act(identb[:], ident[:], AF.Copy)
            fing = cpool.tile([128, CT], F32, tag="fing")
            nc.sync.dma_start(fing[:], d_fing[:])
            finb = cpool.tile([128, CT], F32, tag="finb")
            nc.sync.dma_start(finb[:], d_finb[:])

            # residual stream, channels-major: x[p, ct, tok]
            x = xpool.tile([128, CT, T], F32R, tag="x")

            # ---------------- patch embed ----------------
            with tc.tile_pool(name="patch", bufs=1) as ppool:
                posc = ppool.tile([128, CT, 196], F32, tag="posc")
                nc.sync.dma_start(posc[:],
                                  d_posc.rearrange("(ct p) t -> p ct t", p=128))
                extra = ppool.tile([128, CT, 3], F32, tag="extra")
                nc.sync.dma_start(extra[:],
                                  d_extra.rearrange("(ct p) t -> p ct t", p=128))
                xt = ppool.tile([128, CT, BL * 196], BF16, tag="xt")
                nc.sync.dma_start(xt[:], d_xT.rearrange("(kt p) t -> p kt t", p=128))
                for mt in range(CT):
                    pw = wmtpool.tile([128, CT, 128], BF16, tag="wmt")
                    nc.sync.dma_start(
                        pw[:], d_pw[mt].rearrange("(kt p) m -> p kt m", p=128))
                    for c in range(2):  # 392-token halves: samples (2c, 2c+1)
                        acc = ps.tile([128, 392], F32, tag="ps")
                        for kt in range(CT):
                            mm(acc[:], pw[:, kt, :],
                               xt[:, kt, c * 392:(c + 1) * 392],
                               start=(kt == 0), stop=(kt == CT - 1))
                        for si in range(2):
                            s = 2 * c + si
                            tt(x[:, mt, s * NTOK + 1: s * NTOK + 197],
                               acc[:, si * 196:(si + 1) * 196], posc[:, mt, :],
                               op=OP.add)
                for s in range(BL):
                    o = s * NTOK
                    nc.vector.tensor_copy(x[:, :, o:o + 1], extra[:, :, 0:1])
                    nc.vector.tensor_copy(x[:, :, o + 197:o + 199],
                                          extra[:, :, 1:3])

            # ---------------- transformer blocks ----------------
            for i in range(DEPTH):
                # ===== LN1 -> h (affine folded into qkv weights) =====
                xsq = bigpool.tile([128, CT, T], F32R, tag="big")
                act(xsq[:], x[:], AF.Square)
                h = bigpool.tile([128, CT, T], BF16, tag="big")
                _ln_apply(nc, ps, smpool, stpool, ones1, invc, eps_t, x, xsq, h)

                # ===== qkv = h @ Wqkv (channels-major out) =====
                qk = qkpool.tile([128, 12, T], BF16, tag="qk")
                vT = bigpool.tile([128, CT, T], BF16, tag="big")
                bt = biaspool.tile([128, 54], F32, tag="bt")
                nc.sync.dma_start(bt[:], d_bias[i])
                for mt in range(18):
                    wq = wmtpool.tile([128, CT, 128], BF16, tag="wmt")
                    nc.sync.dma_start(
                        wq[:], d_qkvw[i, mt].rearrange("(kt p) m -> p kt m", p=128))
                    for c in range(2):
                        co, cw = chunk_off(c), CH[c]
                        acc = ps.tile([128, 398], F32, tag="ps")
                        for kt in range(CT):
                            mm(acc[:, :cw], wq[:, kt, :], h[:, kt, co:co + cw],
                               start=(kt == 0), stop=(kt == CT - 1))
                        dst = (qk[:, mt, co:co + cw] if mt < 12
                               else vT[:, mt - 12, co:co + cw])
                        act(dst, acc[:, :cw], AF.Identity, bias=bt[:, mt:mt + 1])

                # ===== transpose v to tokens-major v0/v1 =====
                v0 = vpool.tile([128, BL, DIM], BF16, tag="v0")
                v1 = vpool.tile([128, BL, DIM], BF16, tag="v1")
                for s in range(BL):
                    o = s * NTOK
                    for cc, (vt, mw) in enumerate(((v0, 128), (v1, 71))):
                        for ct in range(CT):
                            ptr = ps.tile([128, 128], BF16, tag="ps")
                            nc.tensor.transpose(
                                ptr[:mw, :],
                                vT[:, ct, o + cc * 128: o + cc * 128 + mw],
                                identb[:])
                            nc.vector.tensor_copy(
                                vt[:mw, s, ct * 128:(ct + 1) * 128],
                                ptr[:mw, :])

                # ===== attention (see _attn_sample) =====
                oT = bigpool.tile([128, CT, T], BF16, tag="big")
                use_mask = i >= MASK_START
                for s in range(BL):
                    _attn_sample(nc, ps, aapool, rrpool, smpool, qk, v0, v1,
                                 oT, onesb, s, use_mask, 0)

                # ===== proj + residual =====
                for mt in range(CT):
                    wp = wmtpool.tile([128, CT, 128], BF16, tag="wmt")
                    nc.sync.dma_start(
                        wp[:], d_prw[i, mt].rearrange("(kt p) m -> p kt m", p=128))
                    for c in range(2):
                        co, cw = chunk_off(c), CH[c]
                        acc = ps.tile([128, 398], F32, tag="ps")
                        for kt in range(CT):
                            mm(acc[:, :cw], wp[:, kt, :], oT[:, kt, co:co + cw],
                               start=(kt == 0), stop=(kt == CT - 1))
                        nc.vector.scalar_tensor_tensor(
                            x[:, mt, co:co + cw], acc[:, :cw],
                            bt[:, 18 + mt:19 + mt], x[:, mt, co:co + cw],
                            op0=OP.add, op1=OP.add)

                # ===== LN2 -> h2 =====
                xsq = bigpool.tile([128, CT, T], F32R, tag="big")
                act(xsq[:], x[:], AF.Square)
                h2 = bigpool.tile([128, CT, T], BF16, tag="big")
                _ln_apply(nc, ps, smpool, stpool, ones1, invc, eps_t, x, xsq, h2)

                # ===== MLP: fc1 -> gelu -> fc2 (streamed, fc2 psum resident) ====
                for c in range(2):
                    co = sum(FCH[:c])
                    cw = FCH[c]
                    fc2acc = []
                    for mt in range(CT):
                        a2 = ps.tile([128, 512], F32, tag="ps")
                        fc2acc.append(a2)
                    for gmt in range(24):
                        wf = wmtpool.tile([128, CT, 128], BF16, tag="wmt")
                        nc.sync.dma_start(
                            wf[:], d_f1w[i, gmt].rearrange("(kt p) m -> p kt m",
                                                           p=128))
                        a1 = ps.tile([128, 512], F32, tag="ps")
                        for kt in range(CT):
                            mm(a1[:, :cw], wf[:, kt, :], h2[:, kt, co:co + cw],
                               start=(kt == 0), stop=(kt == CT - 1))
                        g = gelupool.tile([128, 512], BF16, tag="gelu")
                        act(g[:, :cw], a1[:, :cw], AF.Gelu,
                            bias=bt[:, 30 + gmt:31 + gmt])
                        wf2 = wktpool.tile([128, DIM], BF16, tag="wkt")
                        nc.sync.dma_start(
                            wf2[:], d_f2w[i, gmt * 128:(gmt + 1) * 128, :])
                        for mt in range(CT):
                            mm(fc2acc[mt][:, :cw], wf2[:, mt * 128:(mt + 1) * 128],
                               g[:, :cw], start=(gmt == 0), stop=(gmt == 23))
                    for mt in range(CT):
                        nc.vector.scalar_tensor_tensor(
                            x[:, mt, co:co + cw], fc2acc[mt][:, :cw],
                            bt[:, 24 + mt:25 + mt], x[:, mt, co:co + cw],
                            op0=OP.add, op1=OP.add)

            # -------------- final LN + affine + transpose + out --------------
            xsq = bigpool.tile([128, CT, T], F32R, tag="big")
            act(xsq[:], x[:], AF.Square)
            xfin = bigpool.tile([128, CT, T], F32, tag="big")
            _ln_apply(nc, ps, smpool, stpool, ones1, invc, eps_t, x, xsq, xfin)
            for ct in range(CT):
                act(xfin[:, ct, :], xfin[:, ct, :], AF.Identity,
                    bias=finb[:, ct:ct + 1], scale=fing[:, ct:ct + 1])
            for t in range(7):
                tw = min(128, T - t * 128)
                ob = obpool.tile([128, DIM], F32, tag="ob")
                for ct in range(CT):
                    ptr = ps.tile([128, 128], F32, tag="ps")
                    nc.tensor.transpose(ptr[:tw, :],
                                        xfin[:, ct, t * 128:t * 128 + tw],
                                        ident[:])
                    act(ob[:tw, ct * 128:(ct + 1) * 128], ptr[:tw, :], AF.Copy)
                nc.sync.dma_start(d_out[t * 128:t * 128 + tw, :], ob[:tw, :])

    nc.compile()
    return nc


_CACHED = {}


def _prep_host(inputs):
    """Host-side sharding + layout prep (numpy only)."""
    f = np.float32
    x = inputs["x"]
    qkv_w = np.asarray(inputs["qkv_w"], f)
    ln1_g, ln1_b = np.asarray(inputs["ln1_g"], f), np.asarray(inputs["ln1_b"], f)
    ln2_g, ln2_b = np.asarray(inputs["ln2_g"], f), np.asarray(inputs["ln2_b"], f)
    fc1_w, fc1_b = np.asarray(inputs["fc1_w"], f), np.asarray(inputs["fc1_b"], f)
    fc2_w, fc2_b = np.asarray(inputs["fc2_w"], f), np.asarray(inputs["fc2_b"], f)
    proj_w, proj_b = np.asarray(inputs["proj_w"], f), np.asarray(inputs["proj_b"], f)

    shared = {}
    # patch conv as GEMM: lhsT [cpq, o] split into 6 mt-slices [768, 128]
    pwT = np.asarray(inputs["patch_w"], f).reshape(DIM, 768).T     # [cpq, o]
    shared["patch_wT"] = np.ascontiguousarray(
        pwT.reshape(DIM, CT, 128).transpose(1, 0, 2)).astype(NPBF)
    # qkv weights with ln1 affine fold, mt-sliced
    qkvw = ln1_g[:, :, None] * qkv_w                               # [12,768,2304]
    shared["qkv_wp"] = np.ascontiguousarray(
        qkvw.reshape(12, DIM, 18, 128)[:DEPTH].transpose(0, 2, 1, 3)).astype(NPBF)
    attn_bias = np.einsum("dc,dco->do", ln1_b, qkv_w)              # [12, 2304]
    f1be = fc1_b + np.einsum("dc,dco->do", ln2_b, fc1_w)
    biasT = np.concatenate([
        attn_bias.reshape(12, 18, 128).transpose(0, 2, 1),
        proj_b.reshape(12, 6, 128).transpose(0, 2, 1),
        fc2_b.reshape(12, 6, 128).transpose(0, 2, 1),
        f1be.reshape(12, 24, 128).transpose(0, 2, 1),
    ], axis=2)                                                     # [12, 128, 54]
    shared["biasT"] = np.ascontiguousarray(biasT[:DEPTH]).astype(f)
    shared["proj_wp"] = np.ascontiguousarray(
        proj_w.reshape(12, DIM, CT, 128)[:DEPTH].transpose(0, 2, 1, 3)).astype(NPBF)
    f1w = ln2_g[:, :, None] * fc1_w
    shared["fc1_wp"] = np.ascontiguousarray(
        f1w.reshape(12, DIM, 24, 128)[:DEPTH].transpose(0, 2, 1, 3)).astype(NPBF)
    shared["fc2_w"] = np.ascontiguousarray(fc2_w[:DEPTH]).astype(NPBF)
    shared["posc"] = np.ascontiguousarray(
        (np.asarray(inputs["pos_embed"], f)[0, 1:197]
         + np.asarray(inputs["patch_b"], f)[None, :]).T)
    extra = np.stack([
        np.asarray(inputs["cls_tok"], f)[0, 0] + np.asarray(inputs["pos_embed"], f)[0, 0],
        np.asarray(inputs["loc_tok"], f)[0, 0] + np.asarray(inputs["loc_embed"], f)[0, 0],
        np.asarray(inputs["loc_aug_tok"], f)[0, 0]
        + np.asarray(inputs["loc_aug_embed"], f)[0, 0],
    ], axis=1)
    shared["extra_cols"] = np.ascontiguousarray(extra).astype(f)
    shared["final_g"] = np.ascontiguousarray(
        np.asarray(inputs["norm_g"], f).reshape(CT, 128).T)
    shared["final_b"] = np.ascontiguousarray(
        np.asarray(inputs["norm_b"], f).reshape(CT, 128).T)

    in_maps = []
    for c in range(N_CORES):
        xs = np.asarray(x[BL * c:BL * (c + 1)], f)
        xT = np.ascontiguousarray(
            xs.reshape(BL, 3, 14, 16, 14, 16).transpose(1, 3, 5, 0, 2, 4)
            .reshape(DIM, BL * 196))
        m = dict(shared)
        m["xT"] = xT.astype(NPBF)
        in_maps.append(m)
    return in_maps


def kernel(**inputs):
    if "nc" not in _CACHED:
        _CACHED["nc"] = build_program()
    nc = _CACHED["nc"]
    in_maps = _prep_host(inputs)
    res = run_bass_kernel_spmd(nc, in_maps, list(range(N_CORES)))
    out = np.concatenate([r["out"].reshape(BL, NTOK, DIM) for r in res.results],
                         axis=0)
    return out.astype(np.float32)


# revision 17
# speedup vs baseline: 1.6237x; 1.0986x over previous
"""ViT-style dense transformer (12 blocks, dim 768, 199 tokens, B=32) on 8
Trainium2 NeuronCores.

Sharding: data-parallel over batch — 4 images per core, no collectives.

Device layout: activations are kept channels-major (x.T, shape [768, tokens])
so every GEMM uses the weight as the stationary operand directly. The
residual stream x stays f32r; all GEMM operands (weights, LN outputs,
attention internals, gelu outputs) are bf16: halves the weight-DMA bytes,
runs matmuls at 1 cycle/row for any moving-dim size (so attention needs no
query padding), and halves DVE element time on the 16-bit tensors.
LayerNorm statistics (partition-dim sums) are ones-vector matmuls on the PE;
rstd is exp(-0.5*ln(var+eps)) so only the natural_log_exp + gelu ACT table
sets are used (no sqrt/sigmoid set reloads; the loc-mask sigmoid is computed
from exp). Softmax runs over the partition dim (keys) so the loc-mask is a
per-partition broadcast. LayerNorm affine params are folded into the
following weight matrices host-side; biases enter via activation bias.
"""
import contextlib
import os
import sys

sys.path.insert(0, "/opt/trn_rl_repo")

import ml_dtypes
import numpy as np

import concourse.bass as bass
import concourse.tile as tile
from concourse import bacc, mybir
from concourse.bass_utils import run_bass_kernel_spmd
from concourse.masks import make_identity

F32 = mybir.dt.float32
F32R = mybir.dt.float32r
BF16 = mybir.dt.bfloat16
AF = mybir.ActivationFunctionType
OP = mybir.AluOpType
NPBF = ml_dtypes.bfloat16

N_CORES = 8
BL = 4            # samples per core
DEPTH = int(os.environ.get("KDEPTH", "12"))
MASK_START = int(os.environ.get("KMASKSTART", "9"))
HEADS, DIM, HD = 12, 768, 64
SCALE = HD ** -0.5
NTOK = 199        # tokens per sample
T = BL * NTOK     # 796 tokens per core
CT = DIM // 128   # 6 channel tiles
CH = (398, 398)   # token chunks for N<=512 matmuls
FCH = (512, 284)  # token chunks for the fused MLP (fc2 psum = 6 banks + fc1 2)
EPS = 1e-5


def chunk_off(c):
    return sum(CH[:c])


def _ln_stats(nc, ps, smpool, stpool, ones1, invc, eps_t, x, xsq, c):
    """Per-token mean and rstd (both PSUM [128, cw] broadcasts) over the
    channel (partition x ct) axis of channels-major x, for token chunk c."""
    mm = nc.tensor.matmul
    act = nc.scalar.activation
    tt = nc.vector.tensor_tensor
    co, cw = chunk_off(c), CH[c]
    sraw = ps.tile([1, 398], F32, tag="ps")
    ssraw = ps.tile([1, 398], F32, tag="ps")
    for ct in range(CT):
        mm(sraw[:, :cw], ones1[:, 0:1], x[:, ct, co:co + cw],
           start=(ct == 0), stop=(ct == CT - 1))
        mm(ssraw[:, :cw], ones1[:, 0:1], xsq[:, ct, co:co + cw],
           start=(ct == 0), stop=(ct == CT - 1))
    srow = smpool.tile([1, 2, 398], F32R, tag="srow")
    act(srow[:, 0, :cw], sraw[:, :cw], AF.Copy)
    act(srow[:, 1, :cw], ssraw[:, :cw], AF.Copy)
    mu = ps.tile([128, 398], F32, tag="ps")
    msq = ps.tile([128, 398], F32, tag="ps")
    mm(mu[:, :cw], invc[:], srow[:, 0, :cw], start=True, stop=True)
    mm(msq[:, :cw], invc[:], srow[:, 1, :cw], start=True, stop=True)
    musq = stpool.tile([128, 398], F32, tag="lnsc")
    act(musq[:, :cw], mu[:, :cw], AF.Square)
    var = stpool.tile([128, 398], F32, tag="lnsc")
    tt(var[:, :cw], msq[:, :cw], musq[:, :cw], op=OP.subtract)
    # rstd = exp(-0.5 * ln(var + eps)): stays in the natural_log_exp ACT
    # table set (no sqrt-set reload)
    lv = stpool.tile([128, 398], F32, tag="lnsc")
    act(lv[:, :cw], var[:, :cw], AF.Ln, bias=eps_t[:, 0:1])
    rstd = ps.tile([128, 398], F32, tag="ps")
    act(rstd[:, :cw], lv[:, :cw], AF.Exp, scale=-0.5)
    return mu, rstd


def _ln_apply(nc, ps, smpool, stpool, ones1, invc, eps_t, x, xsq, h):
    """h = (x - mu) * rstd, channels-major, chunk at a time (h may be bf16)."""
    tt = nc.vector.tensor_tensor
    for c in range(2):
        co, cw = chunk_off(c), CH[c]
        mu, rstd = _ln_stats(nc, ps, smpool, stpool, ones1, invc, eps_t,
                             x, xsq, c)
        for ct in range(CT):
            tt(h[:, ct, co:co + cw], x[:, ct, co:co + cw], mu[:, :cw],
               op=OP.subtract)
            tt(h[:, ct, co:co + cw], h[:, ct, co:co + cw], rstd[:, :cw],
               op=OP.mult)


def _attn_sample(nc, ps, aapool, rrpool, smpool, qk, v0, v1, oT, onesb, s,
                 use_mask, first_mask_head):
    """Attention for one sample: scores/softmax/AV, head-pair interleaved.

    aa holds both key chunks per head: cc0 (128 keys) at cols 0:199, cc1
    (71 keys) at cols 256:455. Scores for a head land in ONE psum bank
    (S2 [128,512]); head pairs (rows 0:64 / 64:128 of qk) interleave so
    the PE can overlap their matmuls across row groups, and AV packs each
    pair into one [128,199] psum via col tiling so the copy to
    channels-major oT is partition-aligned (no DMA staging).
    """
    mm = nc.tensor.matmul
    act = nc.scalar.activation
    tt = nc.vector.tensor_tensor
    ts = nc.vector.tensor_scalar
    CO1 = 256
    o = s * NTOK
    aa = aapool.tile([128, 12, 512], BF16, tag="aa")
    macc0 = macc1 = None
    if use_mask:
        macc0 = smpool.tile([128, 2], F32, tag="macc0")
        macc1 = smpool.tile([128, 2], F32, tag="macc1")
    for hp in range(6):
        jq, jk = hp, 6 + hp
        S2s = []
        for idx in range(2):
            ro = idx * 64
            S2 = ps.tile([128, 512], F32, tag="ps")
            S2s.append(S2)
            for cc, mw in ((0, 128), (1, 71)):
                mm(S2[:mw, cc * CO1:cc * CO1 + NTOK],
                   qk[ro:ro + 64, jk, o + cc * 128: o + cc * 128 + mw],
                   qk[ro:ro + 64, jq, o:o + NTOK],
                   start=True, stop=True)
        for idx in range(2):
            h_ = 2 * hp + idx
            S2 = S2s[idx]
            for cc, mw in ((0, 128), (1, 71)):
                act(aa[:mw, h_, cc * CO1:cc * CO1 + NTOK],
                    S2[:mw, cc * CO1:cc * CO1 + NTOK], AF.Exp, scale=SCALE)
                if use_mask:
                    macc = (macc0, macc1)[cc]
                    sl = S2[:mw, cc * CO1 + 197:cc * CO1 + 199]
                    if h_ == 0:
                        nc.vector.tensor_copy(macc[:mw, :], sl)
                    else:
                        tt(macc[:mw, :], macc[:mw, :], sl, op=OP.add)
            ssum = ps.tile([1, NTOK], F32, tag="ps")
            mm(ssum[:, :], onesb[:128, 0:1], aa[:128, h_, 0:NTOK],
               start=True, stop=False)
            mm(ssum[:, :], onesb[:71, 0:1], aa[:71, h_, CO1:CO1 + NTOK],
               start=False, stop=True)
            rr = rrpool.tile([1, NTOK], BF16, tag="rr")
            with nc.allow_low_precision(reason="softmax bf16"):
                nc.vector.reciprocal(rr[:], ssum[:])
            rb = ps.tile([128, NTOK], F32, tag="ps")
            mm(rb[:], onesb[0:1, 0:128], rr[:], start=True, stop=True)
            tt(aa[:, h_, 0:NTOK], aa[:, h_, 0:NTOK], rb[:128, :], op=OP.mult)
            tt(aa[:71, h_, CO1:CO1 + NTOK], aa[:71, h_, CO1:CO1 + NTOK],
               rb[:71, :], op=OP.mult)
    if use_mask:
        for cc, mw in ((0, 128), (1, 71)):
            macc = (macc0, macc1)[cc]
            # sigmoid(z) = 1/(1+exp(-z)) — avoids the sigmoid ACT table set
            sge = smpool.tile([128, 2], F32, tag="sge")
            act(sge[:mw, :], macc[:mw, :], AF.Exp, scale=-SCALE / 12)
            sgd = smpool.tile([128, 2], F32, tag="sgd")
            ts(sgd[:mw, :], sge[:mw, :], 1.0, None, op0=OP.add)
            sg = smpool.tile([128, 2], F32, tag="sg")
            nc.vector.reciprocal(sg[:mw, :], sgd[:mw, :])
            mk = smpool.tile([128, 1], F32, tag="mk")
            tt(mk[:mw, :], sg[:mw, 0:1], sg[:mw, 1:2], op=OP.max)
            coff = cc * CO1
            for h_ in range(12):
                ts(aa[:mw, h_, coff:coff + NTOK],
                   aa[:mw, h_, coff:coff + NTOK], mk[:mw, 0:1], None,
                   op0=OP.mult)
    for hp in range(6):
        OhP = ps.tile([128, NTOK], F32, tag="ps")
        for idx in range(2):
            h_ = 2 * hp + idx
            bp = idx * 64
            mm(OhP[bp:bp + 64, :], v0[:, s, h_ * 64:(h_ + 1) * 64],
               aa[:, h_, 0:NTOK], start=True, stop=False,
               tile_position=(0, bp))
            mm(OhP[bp:bp + 64, :], v1[:71, s, h_ * 64:(h_ + 1) * 64],
               aa[:71, h_, CO1:CO1 + NTOK], start=False, stop=True,
               tile_position=(0, bp))
        act(oT[:, hp, o:o + NTOK], OhP[:, :], AF.Copy)


def build_program():
    nc = bacc.Bacc("TRN2", target_bir_lowering=False, debug=False,
                   num_devices=N_CORES)

    # ---- DRAM parameters (per-core views, host-prepped) ----
    d_xT = nc.dram_tensor("xT", [DIM, BL * 196], BF16, kind="ExternalInput")
    d_pw = nc.dram_tensor("patch_wT", [CT, DIM, 128], BF16, kind="ExternalInput")
    d_qkvw = nc.dram_tensor("qkv_wp", [DEPTH, 18, DIM, 128], BF16,
                            kind="ExternalInput")
    d_bias = nc.dram_tensor("biasT", [DEPTH, 128, 54], F32, kind="ExternalInput")
    d_prw = nc.dram_tensor("proj_wp", [DEPTH, CT, DIM, 128], BF16,
                           kind="ExternalInput")
    d_f1w = nc.dram_tensor("fc1_wp", [DEPTH, 24, DIM, 128], BF16,
                           kind="ExternalInput")
    d_f2w = nc.dram_tensor("fc2_w", [DEPTH, 4 * DIM, DIM], BF16,
                           kind="ExternalInput")
    d_posc = nc.dram_tensor("posc", [DIM, 196], F32, kind="ExternalInput")
    d_extra = nc.dram_tensor("extra_cols", [DIM, 3], F32, kind="ExternalInput")
    d_fing = nc.dram_tensor("final_g", [128, CT], F32, kind="ExternalInput")
    d_finb = nc.dram_tensor("final_b", [128, CT], F32, kind="ExternalInput")
    d_out = nc.dram_tensor("out", [T, DIM], F32, kind="ExternalOutput")

    mm = nc.tensor.matmul
    act = nc.scalar.activation
    tt = nc.vector.tensor_tensor
    ts = nc.vector.tensor_scalar

    KREPEAT = int(os.environ.get("KREPEAT", "1"))
    with tile.TileContext(nc) as tc:
        rep = contextlib.ExitStack()
        if KREPEAT > 1:
            rep.enter_context(tc.For_i(0, KREPEAT, 1))
        with (
            rep,
            tc.tile_pool(name="const", bufs=1) as cpool,
            tc.tile_pool(name="x", bufs=1) as xpool,
            tc.tile_pool(name="big", bufs=2) as bigpool,    # xsq/h/vT/oT/h2 rotate
            tc.tile_pool(name="qk", bufs=1) as qkpool,
            tc.tile_pool(name="v", bufs=1) as vpool,
            tc.tile_pool(name="aa", bufs=2) as aapool,
            tc.tile_pool(name="rr", bufs=2) as rrpool,
            tc.tile_pool(name="wmt", bufs=6) as wmtpool,    # [128,6,128] mt-slices
            tc.tile_pool(name="f2", bufs=1) as f2pool,      # fc2 block weights
            tc.tile_pool(name="bias", bufs=1) as biaspool,
            tc.tile_pool(name="gelu", bufs=3) as gelupool,
            tc.tile_pool(name="stats", bufs=2) as stpool,
            tc.tile_pool(name="small", bufs=1) as smpool,
            tc.tile_pool(name="obuf", bufs=2) as obpool,
            tc.tile_pool(name="ps", bufs=8, space="PSUM") as ps,
        ):
            # ---------------- constants ----------------
            onesf = cpool.tile([128, 128], F32, tag="onesf")
            nc.vector.memset(onesf[:], 1.0)
            onesb = cpool.tile([128, 128], BF16, tag="onesb")
            act(onesb[:], onesf[:], AF.Copy)
            ones1 = cpool.tile([128, 1], F32R, tag="ones1")
            act(ones1[:], onesf[:, 0:1], AF.Copy)
            invc = cpool.tile([1, 128], F32R, tag="invc")
            act(invc[:], onesf[0:1, :], AF.Copy, scale=1.0 / DIM)
            eps_t = cpool.tile([128, 1], F32, tag="eps")
            nc.vector.memset(eps_t[:], EPS)
            ident = cpool.tile([128, 128], F32, tag="ident")
            make_identity(nc, ident[:])
            identb = cpool.tile([128, 128], BF16, tag="identb")
            act(identb[:], ident[:], AF.Copy)
            fing = cpool.tile([128, CT], F32, tag="fing")
            nc.sync.dma_start(fing[:], d_fing[:])
            finb = cpool.tile([128, CT], F32, tag="finb")
            nc.sync.dma_start(finb[:], d_finb[:])

            # residual stream, channels-major: x[p, ct, tok]
            x = xpool.tile([128, CT, T], F32R, tag="x")

            # ---------------- patch embed ----------------
            with tc.tile_pool(name="patch", bufs=1) as ppool:
                posc = ppool.tile([128, CT, 196], F32, tag="posc")
                nc.sync.dma_start(posc[:],
                                  d_posc.rearrange("(ct p) t -> p ct t", p=128))
                extra = ppool.tile([128, CT, 3], F32, tag="extra")
                nc.sync.dma_start(extra[:],
                                  d_extra.rearrange("(ct p) t -> p ct t", p=128))
                xt = ppool.tile([128, CT, BL * 196], BF16, tag="xt")
                nc.sync.dma_start(xt[:], d_xT.rearrange("(kt p) t -> p kt t", p=128))
                for mt in range(CT):
                    pw = wmtpool.tile([128, CT, 128], BF16, tag="wmt")
                    nc.sync.dma_start(
                        pw[:], d_pw[mt].rearrange("(kt p) m -> p kt m", p=128))
                    for c in range(2):  # 392-token halves: samples (2c, 2c+1)
                        acc = ps.tile([128, 392], F32, tag="ps")
                        for kt in range(CT):
                            mm(acc[:], pw[:, kt, :],
                               xt[:, kt, c * 392:(c + 1) * 392],
                               start=(kt == 0), stop=(kt == CT - 1))
                        for si in range(2):
                            s = 2 * c + si
                            tt(x[:, mt, s * NTOK + 1: s * NTOK + 197],
                               acc[:, si * 196:(si + 1) * 196], posc[:, mt, :],
                               op=OP.add)
                for s in range(BL):
                    o = s * NTOK
                    nc.vector.tensor_copy(x[:, :, o:o + 1], extra[:, :, 0:1])
                    nc.vector.tensor_copy(x[:, :, o + 197:o + 199],
                                          extra[:, :, 1:3])

            # ---------------- transformer blocks ----------------
            for i in range(DEPTH):
                # ===== LN1 -> h (affine folded into qkv weights) =====
                xsq = bigpool.tile([128, CT, T], F32R, tag="big")
                act(xsq[:], x[:], AF.Square)
                h = bigpool.tile([128, CT, T], BF16, tag="big")
                _ln_apply(nc, ps, smpool, stpool, ones1, invc, eps_t, x, xsq, h)

                # ===== qkv = h @ Wqkv (channels-major out) =====
                qk = qkpool.tile([128, 12, T], BF16, tag="qk")
                vT = bigpool.tile([128, CT, T], BF16, tag="big")
                bt = biaspool.tile([128, 54], F32, tag="bt")
                nc.sync.dma_start(bt[:], d_bias[i])
                for mt in range(18):
                    wq = wmtpool.tile([128, CT, 128], BF16, tag="wmt")
                    nc.sync.dma_start(
                        wq[:], d_qkvw[i, mt].rearrange("(kt p) m -> p kt m", p=128))
                    for c in range(2):
                        co, cw = chunk_off(c), CH[c]
                        acc = ps.tile([128, 398], F32, tag="ps")
                        for kt in range(CT):
                            mm(acc[:, :cw], wq[:, kt, :], h[:, kt, co:co + cw],
                               start=(kt == 0), stop=(kt == CT - 1))
                        dst = (qk[:, mt, co:co + cw] if mt < 12
                               else vT[:, mt - 12, co:co + cw])
                        act(dst, acc[:, :cw], AF.Identity, bias=bt[:, mt:mt + 1])

                # ===== transpose v to tokens-major v0/v1 =====
                v0 = vpool.tile([128, BL, DIM], BF16, tag="v0")
                v1 = vpool.tile([128, BL, DIM], BF16, tag="v1")
                for s in range(BL):
                    o = s * NTOK
                    for cc, (vt, mw) in enumerate(((v0, 128), (v1, 71))):
                        for ct in range(CT):
                            ptr = ps.tile([128, 128], BF16, tag="ps")
                            nc.tensor.transpose(
                                ptr[:mw, :],
                                vT[:, ct, o + cc * 128: o + cc * 128 + mw],
                                identb[:])
                            nc.vector.tensor_copy(
                                vt[:mw, s, ct * 128:(ct + 1) * 128],
                                ptr[:mw, :])

                # ===== attention (see _attn_sample) =====
                oT = bigpool.tile([128, CT, T], BF16, tag="big")
                use_mask = i >= MASK_START
                for s in range(BL):
                    _attn_sample(nc, ps, aapool, rrpool, smpool, qk, v0, v1,
                                 oT, onesb, s, use_mask, 0)

                # ===== proj + residual =====
                for mt in range(CT):
                    wp = wmtpool.tile([128, CT, 128], BF16, tag="wmt")
                    nc.sync.dma_start(
                        wp[:], d_prw[i, mt].rearrange("(kt p) m -> p kt m", p=128))
                    for c in range(2):
                        co, cw = chunk_off(c), CH[c]
                        acc = ps.tile([128, 398], F32, tag="ps")
                        for kt in range(CT):
                            mm(acc[:, :cw], wp[:, kt, :], oT[:, kt, co:co + cw],
                               start=(kt == 0), stop=(kt == CT - 1))
                        nc.vector.scalar_tensor_tensor(
                            x[:, mt, co:co + cw], acc[:, :cw],
                            bt[:, 18 + mt:19 + mt], x[:, mt, co:co + cw],
                            op0=OP.add, op1=OP.add)

                # ===== LN2 -> h2 =====
                xsq = bigpool.tile([128, CT, T], F32R, tag="big")
                act(xsq[:], x[:], AF.Square)
                h2 = bigpool.tile([128, CT, T], BF16, tag="big")
                _ln_apply(nc, ps, smpool, stpool, ones1, invc, eps_t, x, xsq, h2)

                # ===== MLP: fc1 -> gelu -> fc2 (streamed, fc2 psum resident;
                # fc2 weights SBUF-resident for the block: one big DMA, used
                # by both token chunks) =====
                f2w = f2pool.tile([128, 24, DIM], BF16, tag="f2w")
                nc.sync.dma_start(
                    f2w[:], d_f2w[i].rearrange("(g p) m -> p g m", p=128))
                for c in range(2):
                    co = sum(FCH[:c])
                    cw = FCH[c]
                    fc2acc = []
                    for mt in range(CT):
                        a2 = ps.tile([128, 512], F32, tag="ps")
                        fc2acc.append(a2)
                    for gmt in range(24):
                        wf = wmtpool.tile([128, CT, 128], BF16, tag="wmt")
                        nc.sync.dma_start(
                            wf[:], d_f1w[i, gmt].rearrange("(kt p) m -> p kt m",
                                                           p=128))
                        a1 = ps.tile([128, 512], F32, tag="ps")
                        for kt in range(CT):
                            mm(a1[:, :cw], wf[:, kt, :], h2[:, kt, co:co + cw],
                               start=(kt == 0), stop=(kt == CT - 1))
                        g = gelupool.tile([128, 512], BF16, tag="gelu")
                        act(g[:, :cw], a1[:, :cw], AF.Gelu,
                            bias=bt[:, 30 + gmt:31 + gmt])
                        for mt in range(CT):
                            mm(fc2acc[mt][:, :cw],
                               f2w[:, gmt, mt * 128:(mt + 1) * 128],
                               g[:, :cw], start=(gmt == 0), stop=(gmt == 23))
                    for mt in range(CT):
                        nc.vector.scalar_tensor_tensor(
                            x[:, mt, co:co + cw], fc2acc[mt][:, :cw],
                            bt[:, 24 + mt:25 + mt], x[:, mt, co:co + cw],
                            op0=OP.add, op1=OP.add)

            # -------------- final LN + affine + transpose + out --------------
            xsq = bigpool.tile([128, CT, T], F32R, tag="big")
            act(xsq[:], x[:], AF.Square)
            xfin = bigpool.tile([128, CT, T], F32, tag="big")
            _ln_apply(nc, ps, smpool, stpool, ones1, invc, eps_t, x, xsq, xfin)
            for ct in range(CT):
                act(xfin[:, ct, :], xfin[:, ct, :], AF.Identity,
                    bias=finb[:, ct:ct + 1], scale=fing[:, ct:ct + 1])
            for t in range(7):
                tw = min(128, T - t * 128)
                ob = obpool.tile([128, DIM], F32, tag="ob")
                for ct in range(CT):
                    ptr = ps.tile([128, 128], F32, tag="ps")
                    nc.tensor.transpose(ptr[:tw, :],
                                        xfin[:, ct, t * 128:t * 128 + tw],
                                        ident[:])
                    act(ob[:tw, ct * 128:(ct + 1) * 128], ptr[:tw, :], AF.Copy)
                nc.sync.dma_start(d_out[t * 128:t * 128 + tw, :], ob[:tw, :])

    nc.compile()
    return nc


_CACHED = {}


def _prep_host(inputs):
    """Host-side sharding + layout prep (numpy only)."""
    f = np.float32
    x = inputs["x"]
    qkv_w = np.asarray(inputs["qkv_w"], f)
    ln1_g, ln1_b = np.asarray(inputs["ln1_g"], f), np.asarray(inputs["ln1_b"], f)
    ln2_g, ln2_b = np.asarray(inputs["ln2_g"], f), np.asarray(inputs["ln2_b"], f)
    fc1_w, fc1_b = np.asarray(inputs["fc1_w"], f), np.asarray(inputs["fc1_b"], f)
    fc2_w, fc2_b = np.asarray(inputs["fc2_w"], f), np.asarray(inputs["fc2_b"], f)
    proj_w, proj_b = np.asarray(inputs["proj_w"], f), np.asarray(inputs["proj_b"], f)

    shared = {}
    # patch conv as GEMM: lhsT [cpq, o] split into 6 mt-slices [768, 128]
    pwT = np.asarray(inputs["patch_w"], f).reshape(DIM, 768).T     # [cpq, o]
    shared["patch_wT"] = np.ascontiguousarray(
        pwT.reshape(DIM, CT, 128).transpose(1, 0, 2)).astype(NPBF)
    # qkv weights with ln1 affine fold, mt-sliced
    qkvw = ln1_g[:, :, None] * qkv_w                               # [12,768,2304]
    shared["qkv_wp"] = np.ascontiguousarray(
        qkvw.reshape(12, DIM, 18, 128)[:DEPTH].transpose(0, 2, 1, 3)).astype(NPBF)
    attn_bias = np.einsum("dc,dco->do", ln1_b, qkv_w)              # [12, 2304]
    f1be = fc1_b + np.einsum("dc,dco->do", ln2_b, fc1_w)
    biasT = np.concatenate([
        attn_bias.reshape(12, 18, 128).transpose(0, 2, 1),
        proj_b.reshape(12, 6, 128).transpose(0, 2, 1),
        fc2_b.reshape(12, 6, 128).transpose(0, 2, 1),
        f1be.reshape(12, 24, 128).transpose(0, 2, 1),
    ], axis=2)                                                     # [12, 128, 54]
    shared["biasT"] = np.ascontiguousarray(biasT[:DEPTH]).astype(f)
    shared["proj_wp"] = np.ascontiguousarray(
        proj_w.reshape(12, DIM, CT, 128)[:DEPTH].transpose(0, 2, 1, 3)).astype(NPBF)
    f1w = ln2_g[:, :, None] * fc1_w
    shared["fc1_wp"] = np.ascontiguousarray(
        f1w.reshape(12, DIM, 24, 128)[:DEPTH].transpose(0, 2, 1, 3)).astype(NPBF)
    shared["fc2_w"] = np.ascontiguousarray(fc2_w[:DEPTH]).astype(NPBF)
    shared["posc"] = np.ascontiguousarray(
        (np.asarray(inputs["pos_embed"], f)[0, 1:197]
         + np.asarray(inputs["patch_b"], f)[None, :]).T)
    extra = np.stack([
        np.asarray(inputs["cls_tok"], f)[0, 0] + np.asarray(inputs["pos_embed"], f)[0, 0],
        np.asarray(inputs["loc_tok"], f)[0, 0] + np.asarray(inputs["loc_embed"], f)[0, 0],
        np.asarray(inputs["loc_aug_tok"], f)[0, 0]
        + np.asarray(inputs["loc_aug_embed"], f)[0, 0],
    ], axis=1)
    shared["extra_cols"] = np.ascontiguousarray(extra).astype(f)
    shared["final_g"] = np.ascontiguousarray(
        np.asarray(inputs["norm_g"], f).reshape(CT, 128).T)
    shared["final_b"] = np.ascontiguousarray(
        np.asarray(inputs["norm_b"], f).reshape(CT, 128).T)

    in_maps = []
    for c in range(N_CORES):
        xs = np.asarray(x[BL * c:BL * (c + 1)], f)
        xT = np.ascontiguousarray(
            xs.reshape(BL, 3, 14, 16, 14, 16).transpose(1, 3, 5, 0, 2, 4)
            .reshape(DIM, BL * 196))
        m = dict(shared)
        m["xT"] = xT.astype(NPBF)
        in_maps.append(m)
    return in_maps


def kernel(**inputs):
    if "nc" not in _CACHED:
        _CACHED["nc"] = build_program()
    nc = _CACHED["nc"]
    in_maps = _prep_host(inputs)
    res = run_bass_kernel_spmd(nc, in_maps, list(range(N_CORES)))
    out = np.concatenate([r["out"].reshape(BL, NTOK, DIM) for r in res.results],
                         axis=0)
    return out.astype(np.float32)
